# revision 1
# baseline (speedup 1.0000x reference)
"""GaborNet Trainium2 kernel, v2.

Math per pixel p=(x1,x2), layer l, channel c:
  q_lc(p) = -0.5*||diag(gamma) R (p-mu)||^2   (quadratic in x1,x2)
  s_lc(p) = sin(filt_w . p + filt_b)
  out_0 = exp(q_0)*s_0;  out_l = exp(q_l)*s_l*(W_{l-1} out_{l-1} + b_{l-1})
  final = out_w @ out_4 (+ out_b, applied host-side)

Key structure vs v1:
  * sin is a PE matmul: sin(w.p+b) expanded in a Taylor series around b
    (|w.p| <~ 0.8) into monomials x1^i x2^j of degree <= D=6.  The constant
    term rides in a ones-feature row, so psum_s holds sin() directly and the
    scalar engine never runs Sin -> only Exp+Identity -> ONE act table, zero
    table switches.
  * per-pixel features (exp quadratic feats fp32, sin monomials fp16) are
    built once per core in batch layout [64 groups x slot*C], then gathered
    per tile into matmul K-blocks with one DMA each.
  * fp16 for all SBUF value tensors (DVE 2x mode); fp32r only on the exp-arg
    matmul path for precision.
  * the 9 combine units per tile are routed across DVE (psum TT / stt),
    ACT (identity evac + DVE/Pool mul) and GPSIMD to balance all engines.

Layout: channels on partitions, pixels on free dim; tiles of T=2048 pixels
as 2 groups of C=1024 stacked on partitions (64ch x 2 groups), block-diag
lhsT packing.  Sharding: 8 cores x 65536 consecutive pixels.
"""

import math

import numpy as np

B, DIM, H, W = 2, 2, 512, 512
HID, OUT, NL = 64, 3, 4
NCORES = 8
NPIX = B * H * W // NCORES  # 65536 pixels per core
C = 1024                    # columns per group
NG = NPIX // C              # 64 groups per core
NT = NG // 2                # 32 tiles (2 groups each)
MM = 512                    # psum-bank limit on matmul moving dim
DEG = 5                     # sin Taylor degree

# monomial slot order: (i, j) = x1^i * x2^j.
# slots 0..5 fixed: 1, x1, x2, x1^2, x1*x2, x2^2  (matches exp-feat order
# x1,x2,x1^2,x1x2,x2^2 at slots 1..5 for a contiguous BE->BM copy).
def _monos(deg):
    ms = [(0, 0), (1, 0), (0, 1), (2, 0), (1, 1), (0, 2)]
    for d in range(3, deg + 1):
        for i in range(d, -1, -1):
            ms.append((i, d - i))
    return ms

MONOS = _monos(DEG)
NM = len(MONOS)             # 28
MIDX = {m: k for k, m in enumerate(MONOS)}

# route config (tunable): per-layer sin route and mlp route
#   'dve'      sin: g = TT(psum_s, e)        mlp: out = stt(psum_m,+mb,*g)
#   'act_pool' ACT identity evac -> Pool mul
#   'act_dve'  ACT identity evac -> DVE mul
SIN_ROUTE = ['dve', 'dve', 'act_pool', 'act_pool', 'act_pool']
MLP_ROUTE = [None, 'dve', 'dve', 'dve', 'dve']  # index by layer l (1..4)
FIN_SPLIT = 5               # fin evac: tiles t%FIN_SPLIT==0 -> ACT, else DVE
MONO_POOL_FRAC = 0.5        # fraction of monomial build muls on Pool

_CACHE = {}


def _gabor_coeffs(filt_w, filt_b, mu, gamma, theta):
    """Exp-arg quadratic coeffs on feats [x1,x2,x1^2,x1x2,x2^2] + bias."""
    NL1 = theta.shape[0]
    Ge = np.zeros((NL1, 5, HID), np.float64)
    be = np.zeros((NL1, HID), np.float64)
    for l in range(NL1):
        ang = 2.0 * np.pi * theta[l].astype(np.float64)
        c, s = np.cos(ang), np.sin(ang)
        R = np.stack([np.stack([c, s], -1), np.stack([-s, c], -1)], -2)
        A = gamma[l].astype(np.float64)[:, :, None] * R
        Q = np.einsum('coi,coj->cij', A, A)
        Qmu = np.einsum('cij,cj->ci', Q, mu[l].astype(np.float64))
        Ge[l, 0] = Qmu[:, 0]
        Ge[l, 1] = Qmu[:, 1]
        Ge[l, 2] = -0.5 * Q[:, 0, 0]
        Ge[l, 3] = -Q[:, 0, 1]
        Ge[l, 4] = -0.5 * Q[:, 1, 1]
        be[l] = -0.5 * np.einsum('ci,ci->c', mu[l].astype(np.float64), Qmu)
    return Ge, be


def _sin_poly(filt_w, filt_b):
    """Per layer: [NM, HID] monomial coeffs of sin(w.p + b), Taylor deg DEG."""
    NL1 = filt_b.shape[0]
    P = np.zeros((NL1, NM, HID), np.float64)
    w = filt_w.astype(np.float64)
    b = filt_b.astype(np.float64)
    umax = np.max(np.abs(w[:, :, 0]) + np.abs(w[:, :, 1]))
    assert umax < 1.3, f"sin Taylor deg {DEG} insufficient for |u|max={umax}"
    for l in range(NL1):
        for k in range(DEG + 1):
            dk = np.sin(b[l] + k * np.pi / 2.0) / math.factorial(k)  # [HID]
            for j in range(k + 1):
                m = MIDX[(j, k - j)]
                P[l, m] += dk * math.comb(k, j) * w[l, :, 0] ** j * w[l, :, 1] ** (k - j)
    return P


def _build_consts(filt_w, filt_b, mu, gamma, theta, lin_w, lin_b, out_w, out_b):
    Ge, be = _gabor_coeffs(filt_w, filt_b, mu, gamma, theta)
    P = _sin_poly(filt_w, filt_b)
    NL1 = NL + 1
    # exp lhsT blocks [10, 5*128] f32: rows 0-4 grpA feats, 5-9 grpB
    gabe = np.zeros((10, NL1 * 128), np.float32)
    for l in range(NL1):
        gabe[0:5, l * 128:l * 128 + 64] = Ge[l]
        gabe[5:10, l * 128 + 64:l * 128 + 128] = Ge[l]
    # sin lhsT blocks [2*NM, 5*128] f16
    sint = np.zeros((2 * NM, NL1 * 128), np.float16)
    for l in range(NL1):
        sint[0:NM, l * 128:l * 128 + 64] = P[l]
        sint[NM:2 * NM, l * 128 + 64:l * 128 + 128] = P[l]
    # mlp lhsT [128, 4*128] f16 block-diag W^T
    mlpt = np.zeros((128, NL * 128), np.float16)
    for l in range(NL):
        wT = lin_w[l].T
        mlpt[0:64, l * 128:l * 128 + 64] = wT
        mlpt[64:128, l * 128 + 64:l * 128 + 128] = wT
    # fin lhsT [128, 6] f16
    fint = np.zeros((128, 6), np.float16)
    fint[0:64, 0:3] = out_w.T
    fint[64:128, 3:6] = out_w.T
    beB = np.concatenate([be, be], axis=1).T.astype(np.float32)      # [128,5]
    mbB = np.concatenate([lin_b, lin_b], axis=1).T.astype(np.float32)  # [128,4]
    ones16 = np.ones((NG, C), np.float16)
    obB = np.zeros((128, 1), np.float32)
    obB[0:3, 0] = out_b
    obB[3:6, 0] = out_b
    return dict(gabe=gabe, sint=sint, mlpt=mlpt, fint=fint, be=beB, mb=mbB,
                ones16=ones16, ob=obB)


def _build_nc():
    import concourse.mybir as mybir
    import concourse.tile as tile
    from concourse import bacc

    f32 = mybir.dt.float32
    f32r = mybir.dt.float32r
    f16 = mybir.dt.float16
    AF = mybir.ActivationFunctionType
    ALU = mybir.AluOpType

    nc = bacc.Bacc("TRN2", target_bir_lowering=False, debug=False,
                   enable_asserts=False, num_devices=NCORES)

    xs = nc.dram_tensor("xs", [2, NPIX], f32r, kind="ExternalInput").ap()
    gabe_d = nc.dram_tensor("gabe", [10, 5 * 128], f32r, kind="ExternalInput").ap()
    sint_d = nc.dram_tensor("sint", [2 * NM, 5 * 128], f16, kind="ExternalInput").ap()
    mlpt_d = nc.dram_tensor("mlpt", [128, 4 * 128], f16, kind="ExternalInput").ap()
    fint_d = nc.dram_tensor("fint", [128, 6], f16, kind="ExternalInput").ap()
    be_d = nc.dram_tensor("be", [128, 5], f32, kind="ExternalInput").ap()
    mb_d = nc.dram_tensor("mb", [128, 4], f32, kind="ExternalInput").ap()
    ob_d = nc.dram_tensor("ob", [128, 1], f32, kind="ExternalInput").ap()
    ones_d = nc.dram_tensor("ones16", [NG, C], f16, kind="ExternalInput").ap()
    out_d = nc.dram_tensor("out", [3, NPIX], f32, kind="ExternalOutput").ap()

    def mm2(psum_ap, lhsT_ap, rhs_ap):
        for h in range(C // MM):
            sl = slice(h * MM, (h + 1) * MM)
            nc.tensor.matmul(out=psum_ap[:, sl], lhsT=lhsT_ap,
                             rhs=rhs_ap[:, sl], start=True, stop=True)

    with tile.TileContext(nc) as tc:
        with (
            tc.tile_pool(name="consts", bufs=1) as cpool,
            tc.tile_pool(name="batch", bufs=1) as bpool,
            tc.tile_pool(name="ek", bufs=3) as ekpool,
            tc.tile_pool(name="sk", bufs=3) as skpool,
            tc.tile_pool(name="pq", bufs=2, space="PSUM") as pqpool,
            tc.tile_pool(name="psm", bufs=2, space="PSUM") as psmpool,
            tc.tile_pool(name="e", bufs=12) as epool,
            tc.tile_pool(name="sev", bufs=5) as sevpool,
            tc.tile_pool(name="g", bufs=22) as gpool,
            tc.tile_pool(name="o", bufs=6) as opool,
            tc.tile_pool(name="fin", bufs=3) as fpool,
        ):
            gabe = cpool.tile([10, 5 * 128], f32r)
            nc.sync.dma_start(out=gabe, in_=gabe_d)
            sint = cpool.tile([2 * NM, 5 * 128], f16)
            nc.sync.dma_start(out=sint, in_=sint_d)
            mlpt = cpool.tile([128, 4 * 128], f16)
            nc.sync.dma_start(out=mlpt, in_=mlpt_d)
            fint = cpool.tile([128, 6], f16)
            nc.sync.dma_start(out=fint, in_=fint_d)
            be = cpool.tile([128, 5], f32)
            nc.sync.dma_start(out=be, in_=be_d)
            mb = cpool.tile([128, 4], f32)
            nc.sync.dma_start(out=mb, in_=mb_d)
            ob = cpool.tile([128, 1], f32)
            nc.sync.dma_start(out=ob, in_=ob_d)

            # ---- one-time per-core feature build --------------------------
            # BE [64, 5*C] f32r: x1,x2,x1^2,x1x2,x2^2 per group partition
            BE = bpool.tile([NG, 5 * C], f32r)
            nc.sync.dma_start(
                out=BE[:, 0:C],
                in_=xs[0:1, :].rearrange("c (g p) -> (c g) p", p=C))
            nc.sync.dma_start(
                out=BE[:, C:2 * C],
                in_=xs[1:2, :].rearrange("c (g p) -> (c g) p", p=C))
            nc.vector.tensor_mul(out=BE[:, 2 * C:3 * C], in0=BE[:, 0:C],
                                 in1=BE[:, 0:C])
            nc.vector.tensor_mul(out=BE[:, 3 * C:4 * C], in0=BE[:, 0:C],
                                 in1=BE[:, C:2 * C])
            nc.vector.tensor_mul(out=BE[:, 4 * C:5 * C], in0=BE[:, C:2 * C],
                                 in1=BE[:, C:2 * C])
            # BM [64, NM*C] f16 monomials; slot0 = ones, 1..5 copied from BE
            BM = bpool.tile([NG, NM * C], f16)
            nc.sync.dma_start(out=BM[:, 0:C], in_=ones_d)
            nc.vector.tensor_copy(out=BM[:, C:6 * C], in_=BE[:, 0:5 * C])
            # higher-degree monomials: slot (i,j) = slot(i-1,j)*x1 or
            # slot(i,j-1)*x2
            nmono = 0
            for d in range(3, DEG + 1):
                for i in range(d, -1, -1):
                    j = d - i
                    dst = MIDX[(i, j)]
                    if i > 0:
                        src, mul = MIDX[(i - 1, j)], 1
                    else:
                        src, mul = MIDX[(i, j - 1)], 2
                    eng = nc.gpsimd if (nmono % 2 == 0 and
                                        MONO_POOL_FRAC > 0) else nc.vector
                    eng.tensor_mul(out=BM[:, dst * C:(dst + 1) * C],
                                   in0=BM[:, src * C:(src + 1) * C],
                                   in1=BM[:, mul * C:(mul + 1) * C])
                    nmono += 1

            # ---- main loop: modulo software pipeline ----------------------
            # iteration `it` emits, in this order:
            #   gathers(it+1) | q/exp(it) | s + sin-combine(it-1)
            #   | mlp stage l of tile it-1-l (l=1..4) | fin(it-6)
            # so every instruction's dependencies are >= 1 iteration old and
            # no engine sequencer head-of-line-blocks on a late dependency.
            eK = {}   # t -> expK tile
            sK = {}   # t -> sinK tile
            es = {}   # t -> [e_l]
            gs = {}   # t -> [g_l]
            outs = {} # t -> cur (latest out_l)

            def gather(t):
                expK = ekpool.tile([10, C], f32r, tag="ek")
                nc.sync.dma_start(
                    out=expK,
                    in_=BE[2 * t:2 * t + 2].rearrange("p (f c) -> p f c", c=C))
                sinK = skpool.tile([2 * NM, C], f16, tag="sk")
                nc.sync.dma_start(
                    out=sinK,
                    in_=BM[2 * t:2 * t + 2].rearrange("p (m c) -> p m c", c=C))
                eK[t], sK[t] = expK, sinK

            def qpart(t):
                es[t] = []
                for l in range(5):
                    psQ = pqpool.tile([128, C], f32, tag="q")
                    mm2(psQ, gabe[:, l * 128:(l + 1) * 128], eK[t])
                    e = epool.tile([128, C], f16, tag="e")
                    nc.scalar.activation(out=e, in_=psQ, func=AF.Exp,
                                         bias=be[:, l:l + 1])
                    es[t].append(e)
                del eK[t]

            def spart(t):
                gs[t] = [None] * 5
                for l in (2, 3, 4, 0, 1):
                    pool = pqpool if SIN_ROUTE[l] != 'dve' else psmpool
                    psS = pool.tile([128, C], f32,
                                    tag="q" if SIN_ROUTE[l] != 'dve' else "sm")
                    mm2(psS, sint[:, l * 128:(l + 1) * 128], sK[t])
                    e = es[t][l]
                    route = SIN_ROUTE[l]
                    g = gpool.tile([128, C], f16, tag="g")
                    if route == 'dve':
                        nc.vector.tensor_mul(out=g, in0=psS, in1=e)
                    else:
                        ssb = sevpool.tile([128, C], f16, tag="sev")
                        nc.scalar.activation(out=ssb, in_=psS,
                                             func=AF.Identity, bias=0.0)
                        if route == 'act_pool':
                            nc.gpsimd.tensor_mul(out=g, in0=ssb, in1=e)
                        else:
                            nc.vector.tensor_mul(out=g, in0=ssb, in1=e)
                    gs[t][l] = g
                del es[t], sK[t]
                outs[t] = gs[t][0]

            def mpart(t, l):
                psM = psmpool.tile([128, C], f32, tag="sm")
                mm2(psM, mlpt[:, (l - 1) * 128:l * 128], outs[t])
                mroute = MLP_ROUTE[l]
                nxt = opool.tile([128, C], f16, tag="o")
                if mroute == 'dve':
                    nc.vector.scalar_tensor_tensor(
                        out=nxt, in0=psM, scalar=mb[:, l - 1:l], in1=gs[t][l],
                        op0=ALU.add, op1=ALU.mult)
                else:
                    msb = sevpool.tile([128, C], f16, tag="sev")
                    nc.scalar.activation(out=msb, in_=psM, func=AF.Identity,
                                         bias=mb[:, l - 1:l])
                    if mroute == 'act_pool':
                        nc.gpsimd.tensor_mul(out=nxt, in0=msb, in1=gs[t][l])
                    else:
                        nc.vector.tensor_mul(out=nxt, in0=msb, in1=gs[t][l])
                outs[t] = nxt

            fsbs = {}

            def finpart(t):
                psF = psmpool.tile([128, C], f32, tag="sm")
                mm2(psF[0:6], fint, outs[t])
                fsb = fpool.tile([6, C], f32, tag="f")
                if t % FIN_SPLIT == 0:
                    nc.scalar.activation(out=fsb, in_=psF[0:6],
                                         func=AF.Identity, bias=ob[0:6])
                else:
                    nc.vector.tensor_scalar(out=fsb, in0=psF[0:6],
                                            scalar1=ob[0:6], scalar2=None,
                                            op0=ALU.add)
                fsbs[t] = fsb
                del gs[t], outs[t]

            def outdma(t):
                fsb = fsbs.pop(t)
                nc.sync.dma_start(out=out_d[:, 2 * t * C:(2 * t + 1) * C],
                                  in_=fsb[0:3])
                nc.sync.dma_start(out=out_d[:, (2 * t + 1) * C:(2 * t + 2) * C],
                                  in_=fsb[3:6])

            gather(0)
            for it in range(NT + 8):
                if 0 <= it - 7 < NT:
                    outdma(it - 7)
                if it + 1 < NT:
                    gather(it + 1)
                if it < NT:
                    qpart(it)
                if 0 <= it - 1 < NT:
                    spart(it - 1)
                for l in range(1, 5):
                    if 0 <= it - 1 - l < NT:
                        mpart(it - 1 - l, l)
                if 0 <= it - 6 < NT:
                    finpart(it - 6)
    nc.compile()
    return nc


def _get_nc():
    if "nc" not in _CACHE:
        _CACHE["nc"] = _build_nc()
    return _CACHE["nc"]


def _in_maps(x, consts):
    maps = []
    rows = H // (NCORES // B)  # 128 rows per core
    for k in range(NCORES):
        b, r = k // (NCORES // B), (k % (NCORES // B)) * rows
        m = {"xs": np.ascontiguousarray(
            x[b, :, r:r + rows, :].reshape(2, NPIX), np.float32)}
        m.update(consts)
        maps.append(m)
    return maps


def _assemble(results):
    rows = H // (NCORES // B)
    out = np.empty((B, OUT, H, W), np.float32)
    for k in range(NCORES):
        b, r = k // (NCORES // B), (k % (NCORES // B)) * rows
        out[b, :, r:r + rows, :] = results[k]["out"].reshape(OUT, rows, W)
    return out


def run(x, filt_w, filt_b, mu, gamma, theta, lin_w, lin_b, out_w, out_b,
        trace=False):
    from concourse.bass_utils import run_bass_kernel_spmd
    nc = _get_nc()
    consts = _build_consts(np.asarray(filt_w), np.asarray(filt_b),
                           np.asarray(mu), np.asarray(gamma),
                           np.asarray(theta), np.asarray(lin_w),
                           np.asarray(lin_b), np.asarray(out_w),
                           np.asarray(out_b))
    maps = _in_maps(np.asarray(x), consts)
    res = run_bass_kernel_spmd(nc, maps, core_ids=list(range(NCORES)),
                               trace=trace)
    return _assemble(res.results), res


def kernel(**inputs):
    out, _ = run(**inputs)
    return out



# revision 2
# speedup vs baseline: 1.0205x; 1.0205x over previous
"""GaborNet Trainium2 kernel, v2.

Math per pixel p=(x1,x2), layer l, channel c:
  q_lc(p) = -0.5*||diag(gamma) R (p-mu)||^2   (quadratic in x1,x2)
  s_lc(p) = sin(filt_w . p + filt_b)
  out_0 = exp(q_0)*s_0;  out_l = exp(q_l)*s_l*(W_{l-1} out_{l-1} + b_{l-1})
  final = out_w @ out_4 (+ out_b, applied host-side)

Key structure vs v1:
  * sin is a PE matmul: sin(w.p+b) expanded in a Taylor series around b
    (|w.p| <~ 0.8) into monomials x1^i x2^j of degree <= D=6.  The constant
    term rides in a ones-feature row, so psum_s holds sin() directly and the
    scalar engine never runs Sin -> only Exp+Identity -> ONE act table, zero
    table switches.
  * per-pixel features (exp quadratic feats fp32, sin monomials fp16) are
    built once per core in batch layout [64 groups x slot*C], then gathered
    per tile into matmul K-blocks with one DMA each.
  * fp16 for all SBUF value tensors (DVE 2x mode); fp32r only on the exp-arg
    matmul path for precision.
  * the 9 combine units per tile are routed across DVE (psum TT / stt),
    ACT (identity evac + DVE/Pool mul) and GPSIMD to balance all engines.

Layout: channels on partitions, pixels on free dim; tiles of T=2048 pixels
as 2 groups of C=1024 stacked on partitions (64ch x 2 groups), block-diag
lhsT packing.  Sharding: 8 cores x 65536 consecutive pixels.
"""

import math

import numpy as np

B, DIM, H, W = 2, 2, 512, 512
HID, OUT, NL = 64, 3, 4
NCORES = 8
NPIX = B * H * W // NCORES  # 65536 pixels per core
C = 1024                    # columns per group
NG = NPIX // C              # 64 groups per core
NT = NG // 2                # 32 tiles (2 groups each)
MM = 512                    # psum-bank limit on matmul moving dim
DEG = 5                     # sin Taylor degree

# monomial slot order: (i, j) = x1^i * x2^j.
# slots 0..5 fixed: 1, x1, x2, x1^2, x1*x2, x2^2  (matches exp-feat order
# x1,x2,x1^2,x1x2,x2^2 at slots 1..5 for a contiguous BE->BM copy).
def _monos(deg):
    ms = [(0, 0), (1, 0), (0, 1), (2, 0), (1, 1), (0, 2)]
    for d in range(3, deg + 1):
        for i in range(d, -1, -1):
            ms.append((i, d - i))
    return ms

MONOS = _monos(DEG)
NM = len(MONOS)             # 28
MIDX = {m: k for k, m in enumerate(MONOS)}

# route config (tunable): per-layer sin route and mlp route
#   'dve'      sin: g = TT(psum_s, e)        mlp: out = stt(psum_m,+mb,*g)
#   'act_pool' ACT identity evac -> Pool mul
#   'act_dve'  ACT identity evac -> DVE mul
TAILC = 3                   # tiles with compressed mpart schedule at the end


def sin_route(t, l):
    """Ramp tiles keep ACT lean (exps only); steady evacuates l=2,3,4."""
    if l in (0, 1):
        return 'dve'
    if t < 3:
        return 'dve'
    if t == 3:
        return 'act_pool' if l == 4 else 'dve'
    if t == 4:
        return 'act_pool' if l in (3, 4) else 'dve'
    return 'act_pool'


def mlp_route(t, l):
    return 'dve'


def fin_route(t):
    return 'act' if t >= NT - 5 else 'dve'

_CACHE = {}


def _gabor_coeffs(filt_w, filt_b, mu, gamma, theta):
    """Exp-arg quadratic coeffs on feats [x1,x2,x1^2,x1x2,x2^2] + bias."""
    NL1 = theta.shape[0]
    Ge = np.zeros((NL1, 5, HID), np.float64)
    be = np.zeros((NL1, HID), np.float64)
    for l in range(NL1):
        ang = 2.0 * np.pi * theta[l].astype(np.float64)
        c, s = np.cos(ang), np.sin(ang)
        R = np.stack([np.stack([c, s], -1), np.stack([-s, c], -1)], -2)
        A = gamma[l].astype(np.float64)[:, :, None] * R
        Q = np.einsum('coi,coj->cij', A, A)
        Qmu = np.einsum('cij,cj->ci', Q, mu[l].astype(np.float64))
        Ge[l, 0] = Qmu[:, 0]
        Ge[l, 1] = Qmu[:, 1]
        Ge[l, 2] = -0.5 * Q[:, 0, 0]
        Ge[l, 3] = -Q[:, 0, 1]
        Ge[l, 4] = -0.5 * Q[:, 1, 1]
        be[l] = -0.5 * np.einsum('ci,ci->c', mu[l].astype(np.float64), Qmu)
    return Ge, be


def _sin_poly(filt_w, filt_b):
    """Per layer: [NM, HID] monomial coeffs of sin(w.p + b), Taylor deg DEG."""
    NL1 = filt_b.shape[0]
    P = np.zeros((NL1, NM, HID), np.float64)
    w = filt_w.astype(np.float64)
    b = filt_b.astype(np.float64)
    umax = np.max(np.abs(w[:, :, 0]) + np.abs(w[:, :, 1]))
    assert umax < 1.3, f"sin Taylor deg {DEG} insufficient for |u|max={umax}"
    for l in range(NL1):
        for k in range(DEG + 1):
            dk = np.sin(b[l] + k * np.pi / 2.0) / math.factorial(k)  # [HID]
            for j in range(k + 1):
                m = MIDX[(j, k - j)]
                P[l, m] += dk * math.comb(k, j) * w[l, :, 0] ** j * w[l, :, 1] ** (k - j)
    return P


def _build_consts(filt_w, filt_b, mu, gamma, theta, lin_w, lin_b, out_w, out_b):
    Ge, be = _gabor_coeffs(filt_w, filt_b, mu, gamma, theta)
    P = _sin_poly(filt_w, filt_b)
    NL1 = NL + 1
    # exp lhsT blocks [10, 5*128] f32: rows 0-4 grpA feats, 5-9 grpB
    gabe = np.zeros((10, NL1 * 128), np.float32)
    for l in range(NL1):
        gabe[0:5, l * 128:l * 128 + 64] = Ge[l]
        gabe[5:10, l * 128 + 64:l * 128 + 128] = Ge[l]
    # sin lhsT blocks [2*NM, 5*128] f16
    sint = np.zeros((2 * NM, NL1 * 128), np.float16)
    for l in range(NL1):
        sint[0:NM, l * 128:l * 128 + 64] = P[l]
        sint[NM:2 * NM, l * 128 + 64:l * 128 + 128] = P[l]
    # mlp lhsT [128, 4*128] f16 block-diag W^T
    mlpt = np.zeros((128, NL * 128), np.float16)
    for l in range(NL):
        wT = lin_w[l].T
        mlpt[0:64, l * 128:l * 128 + 64] = wT
        mlpt[64:128, l * 128 + 64:l * 128 + 128] = wT
    # fin lhsT [128, 6] f16
    fint = np.zeros((128, 6), np.float16)
    fint[0:64, 0:3] = out_w.T
    fint[64:128, 3:6] = out_w.T
    beB = np.concatenate([be, be], axis=1).T.astype(np.float32)      # [128,5]
    mbB = np.concatenate([lin_b, lin_b], axis=1).T.astype(np.float32)  # [128,4]
    ones16 = np.ones((NG, C), np.float16)
    obB = np.zeros((128, 1), np.float32)
    obB[0:3, 0] = out_b
    obB[3:6, 0] = out_b
    return dict(gabe=gabe, sint=sint, mlpt=mlpt, fint=fint, be=beB, mb=mbB,
                ones16=ones16, ob=obB)


def _build_nc():
    import concourse.mybir as mybir
    import concourse.tile as tile
    from concourse import bacc

    f32 = mybir.dt.float32
    f32r = mybir.dt.float32r
    f16 = mybir.dt.float16
    AF = mybir.ActivationFunctionType
    ALU = mybir.AluOpType

    nc = bacc.Bacc("TRN2", target_bir_lowering=False, debug=False,
                   enable_asserts=False, num_devices=NCORES)

    xs = nc.dram_tensor("xs", [2, NPIX], f32r, kind="ExternalInput").ap()
    gabe_d = nc.dram_tensor("gabe", [10, 5 * 128], f32r, kind="ExternalInput").ap()
    sint_d = nc.dram_tensor("sint", [2 * NM, 5 * 128], f16, kind="ExternalInput").ap()
    mlpt_d = nc.dram_tensor("mlpt", [128, 4 * 128], f16, kind="ExternalInput").ap()
    fint_d = nc.dram_tensor("fint", [128, 6], f16, kind="ExternalInput").ap()
    be_d = nc.dram_tensor("be", [128, 5], f32, kind="ExternalInput").ap()
    mb_d = nc.dram_tensor("mb", [128, 4], f32, kind="ExternalInput").ap()
    ob_d = nc.dram_tensor("ob", [128, 1], f32, kind="ExternalInput").ap()
    ones_d = nc.dram_tensor("ones16", [NG, C], f16, kind="ExternalInput").ap()
    out_d = nc.dram_tensor("out", [3, NPIX], f32, kind="ExternalOutput").ap()

    def mm2(psum_ap, lhsT_ap, rhs_ap):
        for h in range(C // MM):
            sl = slice(h * MM, (h + 1) * MM)
            nc.tensor.matmul(out=psum_ap[:, sl], lhsT=lhsT_ap,
                             rhs=rhs_ap[:, sl], start=True, stop=True)

    with tile.TileContext(nc) as tc:
        with (
            tc.tile_pool(name="consts", bufs=1) as cpool,
            tc.tile_pool(name="batch", bufs=1) as bpool,
            tc.tile_pool(name="ek", bufs=3) as ekpool,
            tc.tile_pool(name="sk", bufs=3) as skpool,
            tc.tile_pool(name="pq", bufs=2, space="PSUM") as pqpool,
            tc.tile_pool(name="psm", bufs=2, space="PSUM") as psmpool,
            tc.tile_pool(name="e", bufs=12) as epool,
            tc.tile_pool(name="sev", bufs=5) as sevpool,
            tc.tile_pool(name="g", bufs=22) as gpool,
            tc.tile_pool(name="o", bufs=6) as opool,
            tc.tile_pool(name="fin", bufs=3) as fpool,
        ):
            # xs first so the feature build starts ASAP
            BE = bpool.tile([NG, 5 * C], f32r)
            nc.sync.dma_start(
                out=BE[:, 0:C],
                in_=xs[0:1, :].rearrange("c (g p) -> (c g) p", p=C))
            nc.sync.dma_start(
                out=BE[:, C:2 * C],
                in_=xs[1:2, :].rearrange("c (g p) -> (c g) p", p=C))
            gabe = cpool.tile([10, 5 * 128], f32r)
            nc.sync.dma_start(out=gabe, in_=gabe_d)
            be = cpool.tile([128, 5], f32)
            nc.sync.dma_start(out=be, in_=be_d)
            sint = cpool.tile([2 * NM, 5 * 128], f16)
            nc.sync.dma_start(out=sint, in_=sint_d)
            mlpt = cpool.tile([128, 4 * 128], f16)
            nc.sync.dma_start(out=mlpt, in_=mlpt_d)
            fint = cpool.tile([128, 6], f16)
            nc.sync.dma_start(out=fint, in_=fint_d)
            mb = cpool.tile([128, 4], f32)
            nc.sync.dma_start(out=mb, in_=mb_d)
            ob = cpool.tile([128, 1], f32)
            nc.sync.dma_start(out=ob, in_=ob_d)

            # ---- one-time per-core feature build --------------------------
            # BE [64, 5*C] f32r: squares on ACT, cross term on DVE
            nc.scalar.activation(out=BE[:, 2 * C:3 * C], in_=BE[:, 0:C],
                                 func=AF.Square, bias=0.0)
            nc.vector.tensor_mul(out=BE[:, 3 * C:4 * C], in0=BE[:, 0:C],
                                 in1=BE[:, C:2 * C])
            nc.scalar.activation(out=BE[:, 4 * C:5 * C], in_=BE[:, C:2 * C],
                                 func=AF.Square, bias=0.0)
            BM = bpool.tile([NG, NM * C], f16)
            nc.sync.dma_start(out=BM[:, 0:C], in_=ones_d)
            # fp16 base: copy x1,x2; squares/cross recomputed in fp16 (2x DVE)
            nc.vector.tensor_copy(out=BM[:, C:3 * C], in_=BE[:, 0:2 * C])

            def build_monos():
                nc.vector.tensor_mul(out=BM[:, 3 * C:4 * C], in0=BM[:, C:2 * C],
                                     in1=BM[:, C:2 * C])
                nc.vector.tensor_mul(out=BM[:, 4 * C:5 * C], in0=BM[:, C:2 * C],
                                     in1=BM[:, 2 * C:3 * C])
                nc.vector.tensor_mul(out=BM[:, 5 * C:6 * C],
                                     in0=BM[:, 2 * C:3 * C],
                                     in1=BM[:, 2 * C:3 * C])
                nmono = 0
                for d in range(3, DEG + 1):
                    for i in range(d, -1, -1):
                        j = d - i
                        dst = MIDX[(i, j)]
                        if i > 0:
                            src, mul = MIDX[(i - 1, j)], 1
                        else:
                            src, mul = MIDX[(i, j - 1)], 2
                        eng = nc.gpsimd if nmono % 4 == 3 else nc.vector
                        eng.tensor_mul(out=BM[:, dst * C:(dst + 1) * C],
                                       in0=BM[:, src * C:(src + 1) * C],
                                       in1=BM[:, mul * C:(mul + 1) * C])
                        nmono += 1

            # ---- main loop: modulo software pipeline ----------------------
            # iteration `it` emits, in this order:
            #   gathers(it+1) | q/exp(it) | s + sin-combine(it-1)
            #   | mlp stage l of tile it-1-l (l=1..4) | fin(it-6)
            # so every instruction's dependencies are >= 1 iteration old and
            # no engine sequencer head-of-line-blocks on a late dependency.
            eK = {}   # t -> expK tile
            sK = {}   # t -> sinK tile
            es = {}   # t -> [e_l]
            gs = {}   # t -> [g_l]
            outs = {} # t -> cur (latest out_l)

            def gatherE(t):
                expK = ekpool.tile([10, C], f32r, tag="ek")
                nc.sync.dma_start(
                    out=expK,
                    in_=BE[2 * t:2 * t + 2].rearrange("p (f c) -> p f c", c=C))
                eK[t] = expK

            def gatherS(t):
                sinK = skpool.tile([2 * NM, C], f16, tag="sk")
                nc.sync.dma_start(
                    out=sinK,
                    in_=BM[2 * t:2 * t + 2].rearrange("p (m c) -> p m c", c=C))
                sK[t] = sinK

            def gather(t):
                gatherE(t)
                gatherS(t)

            psQs = {}

            def qmm(t):
                psQs[t] = []
                for l in range(5):
                    psQ = pqpool.tile([128, C], f32, tag="q")
                    mm2(psQ, gabe[:, l * 128:(l + 1) * 128], eK[t])
                    psQs[t].append(psQ)
                del eK[t]

            def qexp(t):
                es[t] = []
                for l in range(5):
                    e = epool.tile([128, C], f16, tag="e")
                    nc.scalar.activation(out=e, in_=psQs[t][l], func=AF.Exp,
                                         bias=be[:, l:l + 1])
                    es[t].append(e)
                del psQs[t]

            def qpart(t):
                qmm(t)
                qexp(t)

            def spart(t):
                gs[t] = [None] * 5
                for l in (2, 0, 3, 4, 1):
                    route = sin_route(t, l)
                    pool = pqpool if route != 'dve' else psmpool
                    psS = pool.tile([128, C], f32,
                                    tag="q" if route != 'dve' else "sm")
                    mm2(psS, sint[:, l * 128:(l + 1) * 128], sK[t])
                    e = es[t][l]
                    g = gpool.tile([128, C], f16, tag="g")
                    if route == 'dve':
                        nc.vector.tensor_mul(out=g, in0=psS, in1=e)
                    else:
                        ssb = sevpool.tile([128, C], f16, tag="sev")
                        nc.scalar.activation(out=ssb, in_=psS,
                                             func=AF.Identity, bias=0.0)
                        nc.gpsimd.tensor_mul(out=g, in0=ssb, in1=e)
                    gs[t][l] = g
                del es[t], sK[t]
                outs[t] = gs[t][0]

            def mpart(t, l):
                psM = psmpool.tile([128, C], f32, tag="sm")
                mm2(psM, mlpt[:, (l - 1) * 128:l * 128], outs[t])
                nxt = opool.tile([128, C], f16, tag="o")
                mroute = mlp_route(t, l)
                if mroute == 'dve':
                    nc.vector.scalar_tensor_tensor(
                        out=nxt, in0=psM, scalar=mb[:, l - 1:l], in1=gs[t][l],
                        op0=ALU.add, op1=ALU.mult)
                else:
                    msb = sevpool.tile([128, C], f16, tag="sev")
                    nc.scalar.activation(out=msb, in_=psM, func=AF.Identity,
                                         bias=mb[:, l - 1:l])
                    if mroute == 'act_pool':
                        nc.gpsimd.tensor_mul(out=nxt, in0=msb, in1=gs[t][l])
                    else:
                        nc.vector.tensor_mul(out=nxt, in0=msb, in1=gs[t][l])
                outs[t] = nxt

            fsbs = {}

            def finpart(t):
                psF = psmpool.tile([128, C], f32, tag="sm")
                mm2(psF[0:6], fint, outs[t])
                fsb = fpool.tile([6, C], f32, tag="f")
                if fin_route(t) == 'act':
                    nc.scalar.activation(out=fsb, in_=psF[0:6],
                                         func=AF.Identity, bias=ob[0:6])
                else:
                    nc.vector.tensor_scalar(out=fsb, in0=psF[0:6],
                                            scalar1=ob[0:6], scalar2=None,
                                            op0=ALU.add)
                fsbs[t] = fsb
                del gs[t], outs[t]

            def outdma(t):
                fsb = fsbs.pop(t)
                nc.sync.dma_start(out=out_d[:, 2 * t * C:(2 * t + 1) * C],
                                  in_=fsb[0:3])
                nc.sync.dma_start(out=out_d[:, (2 * t + 1) * C:(2 * t + 2) * C],
                                  in_=fsb[3:6])

            # prologue: tile 0 q-part overlaps the monomial build
            # schedule tables: normal tiles spread mparts over 4 iterations;
            # the last TAILC tiles compress to 2/iteration to shorten drain
            msch = {}   # it -> [(t, l), ...]
            fsch = {}   # it -> [t, ...]
            dsch = {}   # it -> [t, ...]
            for t in range(NT):
                if t < NT - TAILC:
                    for l in range(1, 5):
                        msch.setdefault(t + 1 + l, []).append((t, l))
                    fsch.setdefault(t + 6, []).append(t)
                    dsch.setdefault(t + 7, []).append(t)
                else:
                    msch.setdefault(t + 2, []).extend([(t, 1), (t, 2)])
                    msch.setdefault(t + 3, []).extend([(t, 3), (t, 4)])
                    fsch.setdefault(t + 4, []).append(t)
                    dsch.setdefault(t + 5, []).append(t)
            last_it = max(max(msch), max(fsch), max(dsch))

            gatherE(0)
            qpart(0)
            build_monos()
            gatherS(0)
            gatherE(1)
            gatherS(1)
            qmm(1)
            for it in range(1, last_it + 1):
                for t in dsch.get(it, ()):
                    outdma(t)
                if it + 1 < NT:
                    gatherE(it + 1)
                    gatherS(it + 1)
                if it < NT:
                    qexp(it)
                if 0 <= it - 1 < NT:
                    spart(it - 1)
                for t, l in msch.get(it, ()):
                    mpart(t, l)
                for t in fsch.get(it, ()):
                    finpart(t)
                if it + 1 < NT:
                    qmm(it + 1)
    nc.compile()
    return nc


def _get_nc():
    if "nc" not in _CACHE:
        _CACHE["nc"] = _build_nc()
    return _CACHE["nc"]


def _in_maps(x, consts):
    maps = []
    rows = H // (NCORES // B)  # 128 rows per core
    for k in range(NCORES):
        b, r = k // (NCORES // B), (k % (NCORES // B)) * rows
        m = {"xs": np.ascontiguousarray(
            x[b, :, r:r + rows, :].reshape(2, NPIX), np.float32)}
        m.update(consts)
        maps.append(m)
    return maps


def _assemble(results):
    rows = H // (NCORES // B)
    out = np.empty((B, OUT, H, W), np.float32)
    for k in range(NCORES):
        b, r = k // (NCORES // B), (k % (NCORES // B)) * rows
        out[b, :, r:r + rows, :] = results[k]["out"].reshape(OUT, rows, W)
    return out


def run(x, filt_w, filt_b, mu, gamma, theta, lin_w, lin_b, out_w, out_b,
        trace=False):
    from concourse.bass_utils import run_bass_kernel_spmd
    nc = _get_nc()
    consts = _build_consts(np.asarray(filt_w), np.asarray(filt_b),
                           np.asarray(mu), np.asarray(gamma),
                           np.asarray(theta), np.asarray(lin_w),
                           np.asarray(lin_b), np.asarray(out_w),
                           np.asarray(out_b))
    maps = _in_maps(np.asarray(x), consts)
    res = run_bass_kernel_spmd(nc, maps, core_ids=list(range(NCORES)),
                               trace=trace)
    return _assemble(res.results), res


def kernel(**inputs):
    out, _ = run(**inputs)
    return out



# revision 3
# speedup vs baseline: 1.0417x; 1.0208x over previous
"""GaborNet Trainium2 kernel, v2.

Math per pixel p=(x1,x2), layer l, channel c:
  q_lc(p) = -0.5*||diag(gamma) R (p-mu)||^2   (quadratic in x1,x2)
  s_lc(p) = sin(filt_w . p + filt_b)
  out_0 = exp(q_0)*s_0;  out_l = exp(q_l)*s_l*(W_{l-1} out_{l-1} + b_{l-1})
  final = out_w @ out_4 (+ out_b, applied host-side)

Key structure vs v1:
  * sin is a PE matmul: sin(w.p+b) expanded in a Taylor series around b
    (|w.p| <~ 0.8) into monomials x1^i x2^j of degree <= D=6.  The constant
    term rides in a ones-feature row, so psum_s holds sin() directly and the
    scalar engine never runs Sin -> only Exp+Identity -> ONE act table, zero
    table switches.
  * per-pixel features (exp quadratic feats fp32, sin monomials fp16) are
    built once per core in batch layout [64 groups x slot*C], then gathered
    per tile into matmul K-blocks with one DMA each.
  * fp16 for all SBUF value tensors (DVE 2x mode); fp32r only on the exp-arg
    matmul path for precision.
  * the 9 combine units per tile are routed across DVE (psum TT / stt),
    ACT (identity evac + DVE/Pool mul) and GPSIMD to balance all engines.

Layout: channels on partitions, pixels on free dim; tiles of T=2048 pixels
as 2 groups of C=1024 stacked on partitions (64ch x 2 groups), block-diag
lhsT packing.  Sharding: 8 cores x 65536 consecutive pixels.
"""

import math

import numpy as np

B, DIM, H, W = 2, 2, 512, 512
HID, OUT, NL = 64, 3, 4
NCORES = 8
NPIX = B * H * W // NCORES  # 65536 pixels per core
C = 1024                    # columns per group
NG = NPIX // C              # 64 groups per core
NT = NG // 2                # 32 tiles (2 groups each)
MM = 512                    # psum-bank limit on matmul moving dim
DEG = 5                     # sin Taylor degree

# monomial slot order: (i, j) = x1^i * x2^j.
# slots 0..5 fixed: 1, x1, x2, x1^2, x1*x2, x2^2  (matches exp-feat order
# x1,x2,x1^2,x1x2,x2^2 at slots 1..5 for a contiguous BE->BM copy).
def _monos(deg):
    ms = [(0, 0), (1, 0), (0, 1), (2, 0), (1, 1), (0, 2)]
    for d in range(3, deg + 1):
        for i in range(d, -1, -1):
            ms.append((i, d - i))
    return ms

MONOS = _monos(DEG)
NM = len(MONOS)             # 28
MIDX = {m: k for k, m in enumerate(MONOS)}

# route config (tunable): per-layer sin route and mlp route
#   'dve'      sin: g = TT(psum_s, e)        mlp: out = stt(psum_m,+mb,*g)
#   'act_pool' ACT identity evac -> Pool mul
#   'act_dve'  ACT identity evac -> DVE mul
TAILC = 0                   # tiles with compressed mpart schedule at the end


def sin_route(t, l):
    """Ramp tiles keep ACT lean (exps only); steady evacuates l=2,3,4."""
    if l in (0, 1):
        return 'dve'
    if t < 3:
        return 'dve'
    if t == 3:
        return 'act_pool' if l == 4 else 'dve'
    if t == 4:
        return 'act_pool' if l in (3, 4) else 'dve'
    return 'act_pool'


def mlp_route(t, l):
    return 'dve'


def fin_route(t):
    return 'act' if t >= NT - 9 else 'dve'

_CACHE = {}


def _gabor_coeffs(filt_w, filt_b, mu, gamma, theta):
    """Exp-arg quadratic coeffs on feats [x1,x2,x1^2,x1x2,x2^2] + bias."""
    NL1 = theta.shape[0]
    Ge = np.zeros((NL1, 5, HID), np.float64)
    be = np.zeros((NL1, HID), np.float64)
    for l in range(NL1):
        ang = 2.0 * np.pi * theta[l].astype(np.float64)
        c, s = np.cos(ang), np.sin(ang)
        R = np.stack([np.stack([c, s], -1), np.stack([-s, c], -1)], -2)
        A = gamma[l].astype(np.float64)[:, :, None] * R
        Q = np.einsum('coi,coj->cij', A, A)
        Qmu = np.einsum('cij,cj->ci', Q, mu[l].astype(np.float64))
        Ge[l, 0] = Qmu[:, 0]
        Ge[l, 1] = Qmu[:, 1]
        Ge[l, 2] = -0.5 * Q[:, 0, 0]
        Ge[l, 3] = -Q[:, 0, 1]
        Ge[l, 4] = -0.5 * Q[:, 1, 1]
        be[l] = -0.5 * np.einsum('ci,ci->c', mu[l].astype(np.float64), Qmu)
    return Ge, be


def _sin_poly(filt_w, filt_b):
    """Per layer: [NM, HID] monomial coeffs of sin(w.p + b), Taylor deg DEG."""
    NL1 = filt_b.shape[0]
    P = np.zeros((NL1, NM, HID), np.float64)
    w = filt_w.astype(np.float64)
    b = filt_b.astype(np.float64)
    umax = np.max(np.abs(w[:, :, 0]) + np.abs(w[:, :, 1]))
    assert umax < 1.3, f"sin Taylor deg {DEG} insufficient for |u|max={umax}"
    for l in range(NL1):
        for k in range(DEG + 1):
            dk = np.sin(b[l] + k * np.pi / 2.0) / math.factorial(k)  # [HID]
            for j in range(k + 1):
                m = MIDX[(j, k - j)]
                P[l, m] += dk * math.comb(k, j) * w[l, :, 0] ** j * w[l, :, 1] ** (k - j)
    return P


def _build_consts(filt_w, filt_b, mu, gamma, theta, lin_w, lin_b, out_w, out_b):
    Ge, be = _gabor_coeffs(filt_w, filt_b, mu, gamma, theta)
    P = _sin_poly(filt_w, filt_b)
    NL1 = NL + 1
    # exp lhsT blocks [10, 5*128] f32: rows 0-4 grpA feats, 5-9 grpB
    gabe = np.zeros((10, NL1 * 128), np.float32)
    for l in range(NL1):
        gabe[0:5, l * 128:l * 128 + 64] = Ge[l]
        gabe[5:10, l * 128 + 64:l * 128 + 128] = Ge[l]
    # sin lhsT blocks [2*NM, 5*128] f16
    sint = np.zeros((2 * NM, NL1 * 128), np.float16)
    for l in range(NL1):
        sint[0:NM, l * 128:l * 128 + 64] = P[l]
        sint[NM:2 * NM, l * 128 + 64:l * 128 + 128] = P[l]
    # mlp lhsT [128, 4*128] f16 block-diag W^T
    mlpt = np.zeros((128, NL * 128), np.float16)
    for l in range(NL):
        wT = lin_w[l].T
        mlpt[0:64, l * 128:l * 128 + 64] = wT
        mlpt[64:128, l * 128 + 64:l * 128 + 128] = wT
    # fin lhsT [128, 6] f16
    fint = np.zeros((128, 6), np.float16)
    fint[0:64, 0:3] = out_w.T
    fint[64:128, 3:6] = out_w.T
    beB = np.concatenate([be, be], axis=1).T.astype(np.float32)      # [128,5]
    mbB = np.concatenate([lin_b, lin_b], axis=1).T.astype(np.float32)  # [128,4]
    ones16 = np.ones((NG, C), np.float16)
    obB = np.zeros((128, 1), np.float32)
    obB[0:3, 0] = out_b
    obB[3:6, 0] = out_b
    return dict(gabe=gabe, sint=sint, mlpt=mlpt, fint=fint, be=beB, mb=mbB,
                ones16=ones16, ob=obB)


def _build_nc():
    import concourse.mybir as mybir
    import concourse.tile as tile
    from concourse import bacc

    f32 = mybir.dt.float32
    f32r = mybir.dt.float32r
    f16 = mybir.dt.float16
    AF = mybir.ActivationFunctionType
    ALU = mybir.AluOpType

    nc = bacc.Bacc("TRN2", target_bir_lowering=False, debug=False,
                   enable_asserts=False, num_devices=NCORES)

    xs = nc.dram_tensor("xs", [2, NPIX], f32r, kind="ExternalInput").ap()
    gabe_d = nc.dram_tensor("gabe", [10, 5 * 128], f32r, kind="ExternalInput").ap()
    sint_d = nc.dram_tensor("sint", [2 * NM, 5 * 128], f16, kind="ExternalInput").ap()
    mlpt_d = nc.dram_tensor("mlpt", [128, 4 * 128], f16, kind="ExternalInput").ap()
    fint_d = nc.dram_tensor("fint", [128, 6], f16, kind="ExternalInput").ap()
    be_d = nc.dram_tensor("be", [128, 5], f32, kind="ExternalInput").ap()
    mb_d = nc.dram_tensor("mb", [128, 4], f32, kind="ExternalInput").ap()
    ob_d = nc.dram_tensor("ob", [128, 1], f32, kind="ExternalInput").ap()
    ones_d = nc.dram_tensor("ones16", [NG, C], f16, kind="ExternalInput").ap()
    out_d = nc.dram_tensor("out", [3, NPIX], f32, kind="ExternalOutput").ap()

    def mm2(psum_ap, lhsT_ap, rhs_ap):
        for h in range(C // MM):
            sl = slice(h * MM, (h + 1) * MM)
            nc.tensor.matmul(out=psum_ap[:, sl], lhsT=lhsT_ap,
                             rhs=rhs_ap[:, sl], start=True, stop=True)

    with tile.TileContext(nc) as tc:
        with (
            tc.tile_pool(name="consts", bufs=1) as cpool,
            tc.tile_pool(name="batch", bufs=1) as bpool,
            tc.tile_pool(name="ek", bufs=3) as ekpool,
            tc.tile_pool(name="sk", bufs=3) as skpool,
            tc.tile_pool(name="pq", bufs=2, space="PSUM") as pqpool,
            tc.tile_pool(name="psm", bufs=2, space="PSUM") as psmpool,
            tc.tile_pool(name="e", bufs=12) as epool,
            tc.tile_pool(name="sev", bufs=5) as sevpool,
            tc.tile_pool(name="g", bufs=22) as gpool,
            tc.tile_pool(name="o", bufs=6) as opool,
            tc.tile_pool(name="fin", bufs=3) as fpool,
        ):
            # xs first so the feature build starts ASAP
            BE = bpool.tile([NG, 5 * C], f32r)
            nc.sync.dma_start(
                out=BE[:, 0:C],
                in_=xs[0:1, :].rearrange("c (g p) -> (c g) p", p=C))
            nc.sync.dma_start(
                out=BE[:, C:2 * C],
                in_=xs[1:2, :].rearrange("c (g p) -> (c g) p", p=C))
            gabe = cpool.tile([10, 5 * 128], f32r)
            nc.sync.dma_start(out=gabe, in_=gabe_d)
            be = cpool.tile([128, 5], f32)
            nc.sync.dma_start(out=be, in_=be_d)
            sint = cpool.tile([2 * NM, 5 * 128], f16)
            nc.sync.dma_start(out=sint, in_=sint_d)
            mlpt = cpool.tile([128, 4 * 128], f16)
            nc.sync.dma_start(out=mlpt, in_=mlpt_d)
            fint = cpool.tile([128, 6], f16)
            nc.sync.dma_start(out=fint, in_=fint_d)
            mb = cpool.tile([128, 4], f32)
            nc.sync.dma_start(out=mb, in_=mb_d)
            ob = cpool.tile([128, 1], f32)
            nc.sync.dma_start(out=ob, in_=ob_d)

            # ---- one-time per-core feature build --------------------------
            # BE [64, 5*C] f32r: squares on ACT, cross term on DVE
            nc.scalar.activation(out=BE[:, 2 * C:3 * C], in_=BE[:, 0:C],
                                 func=AF.Square, bias=0.0)
            nc.vector.tensor_mul(out=BE[:, 3 * C:4 * C], in0=BE[:, 0:C],
                                 in1=BE[:, C:2 * C])
            nc.scalar.activation(out=BE[:, 4 * C:5 * C], in_=BE[:, C:2 * C],
                                 func=AF.Square, bias=0.0)
            BM = bpool.tile([NG, NM * C], f16)
            nc.sync.dma_start(out=BM[:, 0:C], in_=ones_d)
            # fp16 base: copy x1,x2; squares/cross recomputed in fp16 (2x DVE)
            nc.vector.tensor_copy(out=BM[:, C:3 * C], in_=BE[:, 0:2 * C])

            def build_monos():
                nc.vector.tensor_mul(out=BM[:, 3 * C:4 * C], in0=BM[:, C:2 * C],
                                     in1=BM[:, C:2 * C])
                nc.vector.tensor_mul(out=BM[:, 4 * C:5 * C], in0=BM[:, C:2 * C],
                                     in1=BM[:, 2 * C:3 * C])
                nc.vector.tensor_mul(out=BM[:, 5 * C:6 * C],
                                     in0=BM[:, 2 * C:3 * C],
                                     in1=BM[:, 2 * C:3 * C])
                nmono = 0
                for d in range(3, DEG + 1):
                    for i in range(d, -1, -1):
                        j = d - i
                        dst = MIDX[(i, j)]
                        if i > 0:
                            src, mul = MIDX[(i - 1, j)], 1
                        else:
                            src, mul = MIDX[(i, j - 1)], 2
                        eng = nc.gpsimd if nmono % 4 == 3 else nc.vector
                        eng.tensor_mul(out=BM[:, dst * C:(dst + 1) * C],
                                       in0=BM[:, src * C:(src + 1) * C],
                                       in1=BM[:, mul * C:(mul + 1) * C])
                        nmono += 1

            # ---- main loop: modulo software pipeline ----------------------
            # iteration `it` emits, in this order:
            #   gathers(it+1) | q/exp(it) | s + sin-combine(it-1)
            #   | mlp stage l of tile it-1-l (l=1..4) | fin(it-6)
            # so every instruction's dependencies are >= 1 iteration old and
            # no engine sequencer head-of-line-blocks on a late dependency.
            eK = {}   # t -> expK tile
            sK = {}   # t -> sinK tile
            es = {}   # t -> [e_l]
            gs = {}   # t -> [g_l]
            outs = {} # t -> cur (latest out_l)

            def gatherE(t):
                expK = ekpool.tile([10, C], f32r, tag="ek")
                nc.sync.dma_start(
                    out=expK,
                    in_=BE[2 * t:2 * t + 2].rearrange("p (f c) -> p f c", c=C))
                eK[t] = expK

            def gatherS(t):
                sinK = skpool.tile([2 * NM, C], f16, tag="sk")
                nc.sync.dma_start(
                    out=sinK,
                    in_=BM[2 * t:2 * t + 2].rearrange("p (m c) -> p m c", c=C))
                sK[t] = sinK

            def gather(t):
                gatherE(t)
                gatherS(t)

            psQs = {}

            def qmm(t):
                psQs[t] = []
                for l in range(5):
                    psQ = pqpool.tile([128, C], f32, tag="q")
                    mm2(psQ, gabe[:, l * 128:(l + 1) * 128], eK[t])
                    psQs[t].append(psQ)
                del eK[t]

            def qexp(t):
                es[t] = []
                for l in range(5):
                    e = epool.tile([128, C], f16, tag="e")
                    nc.scalar.activation(out=e, in_=psQs[t][l], func=AF.Exp,
                                         bias=be[:, l:l + 1])
                    es[t].append(e)
                del psQs[t]

            def qpart(t):
                qmm(t)
                qexp(t)

            def spart(t):
                gs[t] = [None] * 5
                for l in (2, 0, 3, 4, 1):
                    route = sin_route(t, l)
                    pool = pqpool if route != 'dve' else psmpool
                    psS = pool.tile([128, C], f32,
                                    tag="q" if route != 'dve' else "sm")
                    mm2(psS, sint[:, l * 128:(l + 1) * 128], sK[t])
                    e = es[t][l]
                    g = gpool.tile([128, C], f16, tag="g")
                    if route == 'dve':
                        nc.vector.tensor_mul(out=g, in0=psS, in1=e)
                    else:
                        ssb = sevpool.tile([128, C], f16, tag="sev")
                        nc.scalar.activation(out=ssb, in_=psS,
                                             func=AF.Identity, bias=0.0)
                        nc.gpsimd.tensor_mul(out=g, in0=ssb, in1=e)
                    gs[t][l] = g
                del es[t], sK[t]
                outs[t] = gs[t][0]

            def mpart(t, l):
                psM = psmpool.tile([128, C], f32, tag="sm")
                mm2(psM, mlpt[:, (l - 1) * 128:l * 128], outs[t])
                nxt = opool.tile([128, C], f16, tag="o")
                mroute = mlp_route(t, l)
                if mroute == 'dve':
                    nc.vector.scalar_tensor_tensor(
                        out=nxt, in0=psM, scalar=mb[:, l - 1:l], in1=gs[t][l],
                        op0=ALU.add, op1=ALU.mult)
                else:
                    msb = sevpool.tile([128, C], f16, tag="sev")
                    nc.scalar.activation(out=msb, in_=psM, func=AF.Identity,
                                         bias=mb[:, l - 1:l])
                    if mroute == 'act_pool':
                        nc.gpsimd.tensor_mul(out=nxt, in0=msb, in1=gs[t][l])
                    else:
                        nc.vector.tensor_mul(out=nxt, in0=msb, in1=gs[t][l])
                outs[t] = nxt

            fsbs = {}

            def finpart(t):
                psF = psmpool.tile([128, C], f32, tag="sm")
                mm2(psF[0:6], fint, outs[t])
                fsb = fpool.tile([6, C], f32, tag="f")
                if fin_route(t) == 'act':
                    nc.scalar.activation(out=fsb, in_=psF[0:6],
                                         func=AF.Identity, bias=ob[0:6])
                else:
                    nc.vector.tensor_scalar(out=fsb, in0=psF[0:6],
                                            scalar1=ob[0:6], scalar2=None,
                                            op0=ALU.add)
                fsbs[t] = fsb
                del gs[t], outs[t]

            def outdma(t):
                fsb = fsbs.pop(t)
                nc.sync.dma_start(out=out_d[:, 2 * t * C:(2 * t + 1) * C],
                                  in_=fsb[0:3])
                nc.sync.dma_start(out=out_d[:, (2 * t + 1) * C:(2 * t + 2) * C],
                                  in_=fsb[3:6])

            # prologue: tile 0 q-part overlaps the monomial build
            # schedule tables: normal tiles spread mparts over 4 iterations;
            # the last TAILC tiles compress to 2/iteration to shorten drain
            msch = {}   # it -> [(t, l), ...]
            fsch = {}   # it -> [t, ...]
            dsch = {}   # it -> [t, ...]
            for t in range(NT):
                if t < NT - TAILC:
                    for l in range(1, 5):
                        msch.setdefault(t + 1 + l, []).append((t, l))
                    fsch.setdefault(t + 6, []).append(t)
                    dsch.setdefault(t + 7, []).append(t)
                else:
                    msch.setdefault(t + 2, []).extend([(t, 1), (t, 2)])
                    msch.setdefault(t + 3, []).extend([(t, 3), (t, 4)])
                    fsch.setdefault(t + 4, []).append(t)
                    dsch.setdefault(t + 5, []).append(t)
            last_it = max(max(msch), max(fsch), max(dsch))

            gatherE(0)
            qpart(0)
            build_monos()
            gatherS(0)
            gatherE(1)
            gatherS(1)
            qmm(1)
            for it in range(1, last_it + 1):
                for t in dsch.get(it, ()):
                    outdma(t)
                if it + 1 < NT:
                    gatherE(it + 1)
                    gatherS(it + 1)
                if it < NT:
                    qexp(it)
                if 0 <= it - 1 < NT:
                    spart(it - 1)
                for t, l in msch.get(it, ()):
                    mpart(t, l)
                for t in fsch.get(it, ()):
                    finpart(t)
                if it + 1 < NT:
                    qmm(it + 1)
    nc.compile()
    return nc


def _get_nc():
    if "nc" not in _CACHE:
        _CACHE["nc"] = _build_nc()
    return _CACHE["nc"]


def _in_maps(x, consts):
    maps = []
    rows = H // (NCORES // B)  # 128 rows per core
    for k in range(NCORES):
        b, r = k // (NCORES // B), (k % (NCORES // B)) * rows
        m = {"xs": np.ascontiguousarray(
            x[b, :, r:r + rows, :].reshape(2, NPIX), np.float32)}
        m.update(consts)
        maps.append(m)
    return maps


def _assemble(results):
    rows = H // (NCORES // B)
    out = np.empty((B, OUT, H, W), np.float32)
    for k in range(NCORES):
        b, r = k // (NCORES // B), (k % (NCORES // B)) * rows
        out[b, :, r:r + rows, :] = results[k]["out"].reshape(OUT, rows, W)
    return out


def run(x, filt_w, filt_b, mu, gamma, theta, lin_w, lin_b, out_w, out_b,
        trace=False):
    from concourse.bass_utils import run_bass_kernel_spmd
    nc = _get_nc()
    consts = _build_consts(np.asarray(filt_w), np.asarray(filt_b),
                           np.asarray(mu), np.asarray(gamma),
                           np.asarray(theta), np.asarray(lin_w),
                           np.asarray(lin_b), np.asarray(out_w),
                           np.asarray(out_b))
    maps = _in_maps(np.asarray(x), consts)
    res = run_bass_kernel_spmd(nc, maps, core_ids=list(range(NCORES)),
                               trace=trace)
    return _assemble(res.results), res


def kernel(**inputs):
    out, _ = run(**inputs)
    return out



# revision 4
# speedup vs baseline: 1.0421x; 1.0004x over previous
"""GaborNet Trainium2 kernel, v2.

Math per pixel p=(x1,x2), layer l, channel c:
  q_lc(p) = -0.5*||diag(gamma) R (p-mu)||^2   (quadratic in x1,x2)
  s_lc(p) = sin(filt_w . p + filt_b)
  out_0 = exp(q_0)*s_0;  out_l = exp(q_l)*s_l*(W_{l-1} out_{l-1} + b_{l-1})
  final = out_w @ out_4 (+ out_b, applied host-side)

Key structure vs v1:
  * sin is a PE matmul: sin(w.p+b) expanded in a Taylor series around b
    (|w.p| <~ 0.8) into monomials x1^i x2^j of degree <= D=6.  The constant
    term rides in a ones-feature row, so psum_s holds sin() directly and the
    scalar engine never runs Sin -> only Exp+Identity -> ONE act table, zero
    table switches.
  * per-pixel features (exp quadratic feats fp32, sin monomials fp16) are
    built once per core in batch layout [64 groups x slot*C], then gathered
    per tile into matmul K-blocks with one DMA each.
  * fp16 for all SBUF value tensors (DVE 2x mode); fp32r only on the exp-arg
    matmul path for precision.
  * the 9 combine units per tile are routed across DVE (psum TT / stt),
    ACT (identity evac + DVE/Pool mul) and GPSIMD to balance all engines.

Layout: channels on partitions, pixels on free dim; tiles of T=2048 pixels
as 2 groups of C=1024 stacked on partitions (64ch x 2 groups), block-diag
lhsT packing.  Sharding: 8 cores x 65536 consecutive pixels.
"""

import math

import numpy as np

B, DIM, H, W = 2, 2, 512, 512
HID, OUT, NL = 64, 3, 4
NCORES = 8
NPIX = B * H * W // NCORES  # 65536 pixels per core
C = 1024                    # columns per group
NG = NPIX // C              # 64 groups per core
NT = NG // 2                # 32 tiles (2 groups each)
MM = 512                    # psum-bank limit on matmul moving dim
DEG = 5                     # sin Taylor degree

# monomial slot order: (i, j) = x1^i * x2^j.
# slots 0..5 fixed: 1, x1, x2, x1^2, x1*x2, x2^2  (matches exp-feat order
# x1,x2,x1^2,x1x2,x2^2 at slots 1..5 for a contiguous BE->BM copy).
def _monos(deg):
    ms = [(0, 0), (1, 0), (0, 1), (2, 0), (1, 1), (0, 2)]
    for d in range(3, deg + 1):
        for i in range(d, -1, -1):
            ms.append((i, d - i))
    return ms

MONOS = _monos(DEG)
NM = len(MONOS)             # 28
MIDX = {m: k for k, m in enumerate(MONOS)}

# route config (tunable): per-layer sin route and mlp route
#   'dve'      sin: g = TT(psum_s, e)        mlp: out = stt(psum_m,+mb,*g)
#   'act_pool' ACT identity evac -> Pool mul
#   'act_dve'  ACT identity evac -> DVE mul
TAILC = 0                   # tiles with compressed mpart schedule at the end


def sin_route(t, l):
    """Ramp tiles keep ACT lean (exps only); steady evacuates l=2,3,4."""
    if l in (0, 1):
        return 'dve'
    if t < 3:
        return 'dve'
    if t == 3:
        return 'act_pool' if l == 4 else 'dve'
    if t == 4:
        return 'act_pool' if l in (3, 4) else 'dve'
    return 'act_pool'


def mlp_route(t, l):
    return 'dve'


def fin_route(t):
    return 'act' if t >= NT - 9 else 'dve'

_CACHE = {}


def _gabor_coeffs(filt_w, filt_b, mu, gamma, theta):
    """Exp-arg quadratic coeffs on feats [x1,x2,x1^2,x1x2,x2^2] + bias."""
    NL1 = theta.shape[0]
    Ge = np.zeros((NL1, 5, HID), np.float64)
    be = np.zeros((NL1, HID), np.float64)
    for l in range(NL1):
        ang = 2.0 * np.pi * theta[l].astype(np.float64)
        c, s = np.cos(ang), np.sin(ang)
        R = np.stack([np.stack([c, s], -1), np.stack([-s, c], -1)], -2)
        A = gamma[l].astype(np.float64)[:, :, None] * R
        Q = np.einsum('coi,coj->cij', A, A)
        Qmu = np.einsum('cij,cj->ci', Q, mu[l].astype(np.float64))
        Ge[l, 0] = Qmu[:, 0]
        Ge[l, 1] = Qmu[:, 1]
        Ge[l, 2] = -0.5 * Q[:, 0, 0]
        Ge[l, 3] = -Q[:, 0, 1]
        Ge[l, 4] = -0.5 * Q[:, 1, 1]
        be[l] = -0.5 * np.einsum('ci,ci->c', mu[l].astype(np.float64), Qmu)
    return Ge, be


def _sin_poly(filt_w, filt_b):
    """Per layer: [NM, HID] monomial coeffs of sin(w.p + b), Taylor deg DEG."""
    NL1 = filt_b.shape[0]
    P = np.zeros((NL1, NM, HID), np.float64)
    w = filt_w.astype(np.float64)
    b = filt_b.astype(np.float64)
    umax = np.max(np.abs(w[:, :, 0]) + np.abs(w[:, :, 1]))
    assert umax < 1.3, f"sin Taylor deg {DEG} insufficient for |u|max={umax}"
    for l in range(NL1):
        for k in range(DEG + 1):
            dk = np.sin(b[l] + k * np.pi / 2.0) / math.factorial(k)  # [HID]
            for j in range(k + 1):
                m = MIDX[(j, k - j)]
                P[l, m] += dk * math.comb(k, j) * w[l, :, 0] ** j * w[l, :, 1] ** (k - j)
    return P


def _build_consts(filt_w, filt_b, mu, gamma, theta, lin_w, lin_b, out_w, out_b):
    Ge, be = _gabor_coeffs(filt_w, filt_b, mu, gamma, theta)
    P = _sin_poly(filt_w, filt_b)
    NL1 = NL + 1
    # exp lhsT blocks [10, 5*128] f32: rows 0-4 grpA feats, 5-9 grpB
    gabe = np.zeros((10, NL1 * 128), np.float32)
    for l in range(NL1):
        gabe[0:5, l * 128:l * 128 + 64] = Ge[l]
        gabe[5:10, l * 128 + 64:l * 128 + 128] = Ge[l]
    # sin lhsT blocks [2*NM, 5*128] f16
    sint = np.zeros((2 * NM, NL1 * 128), np.float16)
    for l in range(NL1):
        sint[0:NM, l * 128:l * 128 + 64] = P[l]
        sint[NM:2 * NM, l * 128 + 64:l * 128 + 128] = P[l]
    # mlp lhsT [128, 4*128] f16 block-diag W^T
    mlpt = np.zeros((128, NL * 128), np.float16)
    for l in range(NL):
        wT = lin_w[l].T
        mlpt[0:64, l * 128:l * 128 + 64] = wT
        mlpt[64:128, l * 128 + 64:l * 128 + 128] = wT
    # fin lhsT [128, 6] f16
    fint = np.zeros((128, 6), np.float16)
    fint[0:64, 0:3] = out_w.T
    fint[64:128, 3:6] = out_w.T
    beB = np.concatenate([be, be], axis=1).T.astype(np.float32)      # [128,5]
    mbB = np.concatenate([lin_b, lin_b], axis=1).T.astype(np.float32)  # [128,4]
    ones16 = np.ones((NG, C), np.float16)
    obB = np.zeros((128, 1), np.float32)
    obB[0:3, 0] = out_b
    obB[3:6, 0] = out_b
    return dict(gabe=gabe, sint=sint, mlpt=mlpt, fint=fint, be=beB, mb=mbB,
                ones16=ones16, ob=obB)


def _build_nc():
    import concourse.mybir as mybir
    import concourse.tile as tile
    from concourse import bacc

    f32 = mybir.dt.float32
    f32r = mybir.dt.float32r
    f16 = mybir.dt.float16
    AF = mybir.ActivationFunctionType
    ALU = mybir.AluOpType

    nc = bacc.Bacc("TRN2", target_bir_lowering=False, debug=False,
                   enable_asserts=False, num_devices=NCORES)

    xs = nc.dram_tensor("xs", [2, NPIX], f32r, kind="ExternalInput").ap()
    gabe_d = nc.dram_tensor("gabe", [10, 5 * 128], f32r, kind="ExternalInput").ap()
    sint_d = nc.dram_tensor("sint", [2 * NM, 5 * 128], f16, kind="ExternalInput").ap()
    mlpt_d = nc.dram_tensor("mlpt", [128, 4 * 128], f16, kind="ExternalInput").ap()
    fint_d = nc.dram_tensor("fint", [128, 6], f16, kind="ExternalInput").ap()
    be_d = nc.dram_tensor("be", [128, 5], f32, kind="ExternalInput").ap()
    mb_d = nc.dram_tensor("mb", [128, 4], f32, kind="ExternalInput").ap()
    ob_d = nc.dram_tensor("ob", [128, 1], f32, kind="ExternalInput").ap()
    ones_d = nc.dram_tensor("ones16", [NG, C], f16, kind="ExternalInput").ap()
    out_d = nc.dram_tensor("out", [3, NPIX], f32, kind="ExternalOutput").ap()

    def mm2(psum_ap, lhsT_ap, rhs_ap):
        for h in range(C // MM):
            sl = slice(h * MM, (h + 1) * MM)
            nc.tensor.matmul(out=psum_ap[:, sl], lhsT=lhsT_ap,
                             rhs=rhs_ap[:, sl], start=True, stop=True)

    with tile.TileContext(nc) as tc:
        with (
            tc.tile_pool(name="consts", bufs=1) as cpool,
            tc.tile_pool(name="batch", bufs=1) as bpool,
            tc.tile_pool(name="ek", bufs=3) as ekpool,
            tc.tile_pool(name="sk", bufs=3) as skpool,
            tc.tile_pool(name="pq", bufs=2, space="PSUM") as pqpool,
            tc.tile_pool(name="psm", bufs=2, space="PSUM") as psmpool,
            tc.tile_pool(name="e", bufs=12) as epool,
            tc.tile_pool(name="sev", bufs=5) as sevpool,
            tc.tile_pool(name="g", bufs=22) as gpool,
            tc.tile_pool(name="o", bufs=6) as opool,
            tc.tile_pool(name="fin", bufs=3) as fpool,
        ):
            # xs first so the feature build starts ASAP
            BE = bpool.tile([NG, 5 * C], f32r)
            nc.sync.dma_start(
                out=BE[:, 0:C],
                in_=xs[0:1, :].rearrange("c (g p) -> (c g) p", p=C))
            nc.sync.dma_start(
                out=BE[:, C:2 * C],
                in_=xs[1:2, :].rearrange("c (g p) -> (c g) p", p=C))
            gabe = cpool.tile([10, 5 * 128], f32r)
            nc.sync.dma_start(out=gabe, in_=gabe_d)
            be = cpool.tile([128, 5], f32)
            nc.sync.dma_start(out=be, in_=be_d)
            sint = cpool.tile([2 * NM, 5 * 128], f16)
            nc.sync.dma_start(out=sint, in_=sint_d)
            mlpt = cpool.tile([128, 4 * 128], f16)
            nc.sync.dma_start(out=mlpt, in_=mlpt_d)
            fint = cpool.tile([128, 6], f16)
            nc.sync.dma_start(out=fint, in_=fint_d)
            mb = cpool.tile([128, 4], f32)
            nc.sync.dma_start(out=mb, in_=mb_d)
            ob = cpool.tile([128, 1], f32)
            nc.sync.dma_start(out=ob, in_=ob_d)

            # ---- one-time per-core feature build --------------------------
            # BE [64, 5*C] f32r: squares on ACT, cross term on DVE
            nc.scalar.activation(out=BE[:, 2 * C:3 * C], in_=BE[:, 0:C],
                                 func=AF.Square, bias=0.0)
            nc.vector.tensor_mul(out=BE[:, 3 * C:4 * C], in0=BE[:, 0:C],
                                 in1=BE[:, C:2 * C])
            nc.scalar.activation(out=BE[:, 4 * C:5 * C], in_=BE[:, C:2 * C],
                                 func=AF.Square, bias=0.0)
            BM = bpool.tile([NG, NM * C], f16)
            nc.sync.dma_start(out=BM[:, 0:C], in_=ones_d)
            # fp16 base: copy x1,x2; squares/cross recomputed in fp16 (2x DVE)
            nc.vector.tensor_copy(out=BM[:, C:3 * C], in_=BE[:, 0:2 * C])

            def build_monos():
                nc.vector.tensor_mul(out=BM[:, 3 * C:4 * C], in0=BM[:, C:2 * C],
                                     in1=BM[:, C:2 * C])
                nc.vector.tensor_mul(out=BM[:, 4 * C:5 * C], in0=BM[:, C:2 * C],
                                     in1=BM[:, 2 * C:3 * C])
                nc.vector.tensor_mul(out=BM[:, 5 * C:6 * C],
                                     in0=BM[:, 2 * C:3 * C],
                                     in1=BM[:, 2 * C:3 * C])
                nmono = 0
                for d in range(3, DEG + 1):
                    for i in range(d, -1, -1):
                        j = d - i
                        dst = MIDX[(i, j)]
                        if i > 0:
                            src, mul = MIDX[(i - 1, j)], 1
                        else:
                            src, mul = MIDX[(i, j - 1)], 2
                        eng = nc.gpsimd if nmono % 4 == 3 else nc.vector
                        eng.tensor_mul(out=BM[:, dst * C:(dst + 1) * C],
                                       in0=BM[:, src * C:(src + 1) * C],
                                       in1=BM[:, mul * C:(mul + 1) * C])
                        nmono += 1

            # ---- main loop: modulo software pipeline ----------------------
            # iteration `it` emits, in this order:
            #   gathers(it+1) | q/exp(it) | s + sin-combine(it-1)
            #   | mlp stage l of tile it-1-l (l=1..4) | fin(it-6)
            # so every instruction's dependencies are >= 1 iteration old and
            # no engine sequencer head-of-line-blocks on a late dependency.
            eK = {}   # t -> expK tile
            sK = {}   # t -> sinK tile
            es = {}   # t -> [e_l]
            gs = {}   # t -> [g_l]
            outs = {} # t -> cur (latest out_l)

            def gatherE(t):
                expK = ekpool.tile([10, C], f32r, tag="ek")
                nc.sync.dma_start(
                    out=expK,
                    in_=BE[2 * t:2 * t + 2].rearrange("p (f c) -> p f c", c=C))
                eK[t] = expK

            def gatherS(t):
                sinK = skpool.tile([2 * NM, C], f16, tag="sk")
                nc.sync.dma_start(
                    out=sinK,
                    in_=BM[2 * t:2 * t + 2].rearrange("p (m c) -> p m c", c=C))
                sK[t] = sinK

            def gather(t):
                gatherE(t)
                gatherS(t)

            psQs = {}

            def qmm(t):
                psQs[t] = []
                for l in range(5):
                    psQ = pqpool.tile([128, C], f32, tag="q")
                    mm2(psQ, gabe[:, l * 128:(l + 1) * 128], eK[t])
                    psQs[t].append(psQ)
                del eK[t]

            def qexp(t):
                es[t] = []
                for l in range(5):
                    e = epool.tile([128, C], f16, tag="e")
                    nc.scalar.activation(out=e, in_=psQs[t][l], func=AF.Exp,
                                         bias=be[:, l:l + 1])
                    es[t].append(e)
                del psQs[t]

            def qpart(t):
                qmm(t)
                qexp(t)

            def spart(t):
                gs[t] = [None] * 5
                for l in (2, 0, 3, 1, 4):
                    route = sin_route(t, l)
                    pool = pqpool if route != 'dve' else psmpool
                    psS = pool.tile([128, C], f32,
                                    tag="q" if route != 'dve' else "sm")
                    mm2(psS, sint[:, l * 128:(l + 1) * 128], sK[t])
                    e = es[t][l]
                    g = gpool.tile([128, C], f16, tag="g")
                    if route == 'dve':
                        nc.vector.tensor_mul(out=g, in0=psS, in1=e)
                    else:
                        ssb = sevpool.tile([128, C], f16, tag="sev")
                        nc.scalar.activation(out=ssb, in_=psS,
                                             func=AF.Identity, bias=0.0)
                        nc.gpsimd.tensor_mul(out=g, in0=ssb, in1=e)
                    gs[t][l] = g
                del es[t], sK[t]
                outs[t] = gs[t][0]

            def mpart(t, l):
                psM = psmpool.tile([128, C], f32, tag="sm")
                mm2(psM, mlpt[:, (l - 1) * 128:l * 128], outs[t])
                nxt = opool.tile([128, C], f16, tag="o")
                mroute = mlp_route(t, l)
                if mroute == 'dve':
                    nc.vector.scalar_tensor_tensor(
                        out=nxt, in0=psM, scalar=mb[:, l - 1:l], in1=gs[t][l],
                        op0=ALU.add, op1=ALU.mult)
                else:
                    msb = sevpool.tile([128, C], f16, tag="sev")
                    nc.scalar.activation(out=msb, in_=psM, func=AF.Identity,
                                         bias=mb[:, l - 1:l])
                    if mroute == 'act_pool':
                        nc.gpsimd.tensor_mul(out=nxt, in0=msb, in1=gs[t][l])
                    else:
                        nc.vector.tensor_mul(out=nxt, in0=msb, in1=gs[t][l])
                outs[t] = nxt

            fsbs = {}

            def finpart(t):
                psF = psmpool.tile([128, C], f32, tag="sm")
                mm2(psF[0:6], fint, outs[t])
                fsb = fpool.tile([6, C], f32, tag="f")
                if fin_route(t) == 'act':
                    nc.scalar.activation(out=fsb, in_=psF[0:6],
                                         func=AF.Identity, bias=ob[0:6])
                else:
                    nc.vector.tensor_scalar(out=fsb, in0=psF[0:6],
                                            scalar1=ob[0:6], scalar2=None,
                                            op0=ALU.add)
                fsbs[t] = fsb
                del gs[t], outs[t]

            def outdma(t):
                fsb = fsbs.pop(t)
                nc.sync.dma_start(out=out_d[:, 2 * t * C:(2 * t + 1) * C],
                                  in_=fsb[0:3])
                nc.sync.dma_start(out=out_d[:, (2 * t + 1) * C:(2 * t + 2) * C],
                                  in_=fsb[3:6])

            # prologue: tile 0 q-part overlaps the monomial build
            # schedule tables: normal tiles spread mparts over 4 iterations;
            # the last TAILC tiles compress to 2/iteration to shorten drain
            msch = {}   # it -> [(t, l), ...]
            fsch = {}   # it -> [t, ...]
            dsch = {}   # it -> [t, ...]
            for t in range(NT):
                if t < NT - TAILC:
                    for l in range(1, 5):
                        msch.setdefault(t + 1 + l, []).append((t, l))
                    fsch.setdefault(t + 6, []).append(t)
                    dsch.setdefault(t + 7, []).append(t)
                else:
                    msch.setdefault(t + 2, []).extend([(t, 1), (t, 2)])
                    msch.setdefault(t + 3, []).extend([(t, 3), (t, 4)])
                    fsch.setdefault(t + 4, []).append(t)
                    dsch.setdefault(t + 5, []).append(t)
            last_it = max(max(msch), max(fsch), max(dsch))

            gatherE(0)
            qpart(0)
            build_monos()
            gatherS(0)
            gatherE(1)
            gatherS(1)
            qmm(1)
            for it in range(1, last_it + 1):
                for t in dsch.get(it, ()):
                    outdma(t)
                if it + 1 < NT:
                    gatherE(it + 1)
                    gatherS(it + 1)
                if it < NT:
                    qexp(it)
                if 0 <= it - 1 < NT:
                    spart(it - 1)
                for t, l in msch.get(it, ()):
                    mpart(t, l)
                for t in fsch.get(it, ()):
                    finpart(t)
                if it + 1 < NT:
                    qmm(it + 1)
    nc.compile()
    return nc


def _get_nc():
    if "nc" not in _CACHE:
        _CACHE["nc"] = _build_nc()
    return _CACHE["nc"]


def _in_maps(x, consts):
    maps = []
    rows = H // (NCORES // B)  # 128 rows per core
    for k in range(NCORES):
        b, r = k // (NCORES // B), (k % (NCORES // B)) * rows
        m = {"xs": np.ascontiguousarray(
            x[b, :, r:r + rows, :].reshape(2, NPIX), np.float32)}
        m.update(consts)
        maps.append(m)
    return maps


def _assemble(results):
    rows = H // (NCORES // B)
    out = np.empty((B, OUT, H, W), np.float32)
    for k in range(NCORES):
        b, r = k // (NCORES // B), (k % (NCORES // B)) * rows
        out[b, :, r:r + rows, :] = results[k]["out"].reshape(OUT, rows, W)
    return out


def run(x, filt_w, filt_b, mu, gamma, theta, lin_w, lin_b, out_w, out_b,
        trace=False):
    from concourse.bass_utils import run_bass_kernel_spmd
    nc = _get_nc()
    consts = _build_consts(np.asarray(filt_w), np.asarray(filt_b),
                           np.asarray(mu), np.asarray(gamma),
                           np.asarray(theta), np.asarray(lin_w),
                           np.asarray(lin_b), np.asarray(out_w),
                           np.asarray(out_b))
    maps = _in_maps(np.asarray(x), consts)
    res = run_bass_kernel_spmd(nc, maps, core_ids=list(range(NCORES)),
                               trace=trace)
    return _assemble(res.results), res


def kernel(**inputs):
    out, _ = run(**inputs)
    return out



# revision 5
# speedup vs baseline: 1.0475x; 1.0052x over previous
"""GaborNet Trainium2 kernel, v2.

Math per pixel p=(x1,x2), layer l, channel c:
  q_lc(p) = -0.5*||diag(gamma) R (p-mu)||^2   (quadratic in x1,x2)
  s_lc(p) = sin(filt_w . p + filt_b)
  out_0 = exp(q_0)*s_0;  out_l = exp(q_l)*s_l*(W_{l-1} out_{l-1} + b_{l-1})
  final = out_w @ out_4 (+ out_b, applied host-side)

Key structure vs v1:
  * sin is a PE matmul: sin(w.p+b) expanded in a Taylor series around b
    (|w.p| <~ 0.8) into monomials x1^i x2^j of degree <= D=6.  The constant
    term rides in a ones-feature row, so psum_s holds sin() directly and the
    scalar engine never runs Sin -> only Exp+Identity -> ONE act table, zero
    table switches.
  * per-pixel features (exp quadratic feats fp32, sin monomials fp16) are
    built once per core in batch layout [64 groups x slot*C], then gathered
    per tile into matmul K-blocks with one DMA each.
  * fp16 for all SBUF value tensors (DVE 2x mode); fp32r only on the exp-arg
    matmul path for precision.
  * the 9 combine units per tile are routed across DVE (psum TT / stt),
    ACT (identity evac + DVE/Pool mul) and GPSIMD to balance all engines.

Layout: channels on partitions, pixels on free dim; tiles of T=2048 pixels
as 2 groups of C=1024 stacked on partitions (64ch x 2 groups), block-diag
lhsT packing.  Sharding: 8 cores x 65536 consecutive pixels.
"""

import math

import numpy as np

B, DIM, H, W = 2, 2, 512, 512
HID, OUT, NL = 64, 3, 4
NCORES = 8
NPIX = B * H * W // NCORES  # 65536 pixels per core
C = 1024                    # columns per group
NG = NPIX // C              # 64 groups per core
NT = NG // 2                # 32 tiles (2 groups each)
MM = 512                    # psum-bank limit on matmul moving dim
DEG = 5                     # sin Taylor degree

# monomial slot order: (i, j) = x1^i * x2^j.
# slots 0..5 fixed: 1, x1, x2, x1^2, x1*x2, x2^2  (matches exp-feat order
# x1,x2,x1^2,x1x2,x2^2 at slots 1..5 for a contiguous BE->BM copy).
def _monos(deg):
    ms = [(0, 0), (1, 0), (0, 1), (2, 0), (1, 1), (0, 2)]
    for d in range(3, deg + 1):
        for i in range(d, -1, -1):
            ms.append((i, d - i))
    return ms

MONOS = _monos(DEG)
NM = len(MONOS)             # 28
MIDX = {m: k for k, m in enumerate(MONOS)}

# route config (tunable): per-layer sin route and mlp route
#   'dve'      sin: g = TT(psum_s, e)        mlp: out = stt(psum_m,+mb,*g)
#   'act_pool' ACT identity evac -> Pool mul
#   'act_dve'  ACT identity evac -> DVE mul
TAILC = 0                   # tiles with compressed mpart schedule at the end


def sin_route(t, l):
    """Ramp tiles keep ACT lean (exps only); steady evacuates l=2,3,4."""
    if l in (0, 1):
        return 'dve'
    if t < 3:
        return 'dve'
    if t == 3:
        return 'act_pool' if l == 4 else 'dve'
    if t == 4:
        return 'act_pool' if l in (3, 4) else 'dve'
    return 'act_pool'


def mlp_route(t, l):
    return 'dve'


def fin_route(t):
    return 'act' if t >= NT - 9 else 'dve'

_CACHE = {}


def _gabor_coeffs(filt_w, filt_b, mu, gamma, theta):
    """Exp-arg quadratic coeffs on feats [x1,x2,x1^2,x1x2,x2^2] + bias."""
    NL1 = theta.shape[0]
    Ge = np.zeros((NL1, 5, HID), np.float64)
    be = np.zeros((NL1, HID), np.float64)
    for l in range(NL1):
        ang = 2.0 * np.pi * theta[l].astype(np.float64)
        c, s = np.cos(ang), np.sin(ang)
        R = np.stack([np.stack([c, s], -1), np.stack([-s, c], -1)], -2)
        A = gamma[l].astype(np.float64)[:, :, None] * R
        Q = np.einsum('coi,coj->cij', A, A)
        Qmu = np.einsum('cij,cj->ci', Q, mu[l].astype(np.float64))
        Ge[l, 0] = Qmu[:, 0]
        Ge[l, 1] = Qmu[:, 1]
        Ge[l, 2] = -0.5 * Q[:, 0, 0]
        Ge[l, 3] = -Q[:, 0, 1]
        Ge[l, 4] = -0.5 * Q[:, 1, 1]
        be[l] = -0.5 * np.einsum('ci,ci->c', mu[l].astype(np.float64), Qmu)
    return Ge, be


def _sin_poly(filt_w, filt_b):
    """Per layer: [NM, HID] monomial coeffs of sin(w.p + b), Taylor deg DEG."""
    NL1 = filt_b.shape[0]
    P = np.zeros((NL1, NM, HID), np.float64)
    w = filt_w.astype(np.float64)
    b = filt_b.astype(np.float64)
    umax = np.max(np.abs(w[:, :, 0]) + np.abs(w[:, :, 1]))
    assert umax < 1.3, f"sin Taylor deg {DEG} insufficient for |u|max={umax}"
    for l in range(NL1):
        for k in range(DEG + 1):
            dk = np.sin(b[l] + k * np.pi / 2.0) / math.factorial(k)  # [HID]
            for j in range(k + 1):
                m = MIDX[(j, k - j)]
                P[l, m] += dk * math.comb(k, j) * w[l, :, 0] ** j * w[l, :, 1] ** (k - j)
    return P


def _build_consts(filt_w, filt_b, mu, gamma, theta, lin_w, lin_b, out_w, out_b):
    Ge, be = _gabor_coeffs(filt_w, filt_b, mu, gamma, theta)
    P = _sin_poly(filt_w, filt_b)
    NL1 = NL + 1
    # exp lhsT blocks [10, 5*128] f32: rows 0-4 grpA feats, 5-9 grpB
    gabe = np.zeros((10, NL1 * 128), np.float32)
    for l in range(NL1):
        gabe[0:5, l * 128:l * 128 + 64] = Ge[l]
        gabe[5:10, l * 128 + 64:l * 128 + 128] = Ge[l]
    # sin lhsT blocks [2*NM, 5*128] f16
    sint = np.zeros((2 * NM, NL1 * 128), np.float16)
    for l in range(NL1):
        sint[0:NM, l * 128:l * 128 + 64] = P[l]
        sint[NM:2 * NM, l * 128 + 64:l * 128 + 128] = P[l]
    # mlp lhsT [128, 4*128] f16 block-diag W^T
    mlpt = np.zeros((128, NL * 128), np.float16)
    for l in range(NL):
        wT = lin_w[l].T
        mlpt[0:64, l * 128:l * 128 + 64] = wT
        mlpt[64:128, l * 128 + 64:l * 128 + 128] = wT
    # fin lhsT [128, 6] f16
    fint = np.zeros((128, 6), np.float16)
    fint[0:64, 0:3] = out_w.T
    fint[64:128, 3:6] = out_w.T
    beB = np.concatenate([be, be], axis=1).T.astype(np.float32)      # [128,5]
    mbB = np.concatenate([lin_b, lin_b], axis=1).T.astype(np.float32)  # [128,4]
    ones16 = np.ones((NG, C), np.float16)
    obB = np.zeros((128, 1), np.float32)
    obB[0:3, 0] = out_b
    obB[3:6, 0] = out_b
    return dict(gabe=gabe, sint=sint, mlpt=mlpt, fint=fint, be=beB, mb=mbB,
                ones16=ones16, ob=obB)


def _build_nc():
    import concourse.mybir as mybir
    import concourse.tile as tile
    from concourse import bacc

    f32 = mybir.dt.float32
    f32r = mybir.dt.float32r
    f16 = mybir.dt.float16
    AF = mybir.ActivationFunctionType
    ALU = mybir.AluOpType

    nc = bacc.Bacc("TRN2", target_bir_lowering=False, debug=False,
                   enable_asserts=False, num_devices=NCORES)

    xs = nc.dram_tensor("xs", [2, NPIX], f32r, kind="ExternalInput").ap()
    gabe_d = nc.dram_tensor("gabe", [10, 5 * 128], f32r, kind="ExternalInput").ap()
    sint_d = nc.dram_tensor("sint", [2 * NM, 5 * 128], f16, kind="ExternalInput").ap()
    mlpt_d = nc.dram_tensor("mlpt", [128, 4 * 128], f16, kind="ExternalInput").ap()
    fint_d = nc.dram_tensor("fint", [128, 6], f16, kind="ExternalInput").ap()
    be_d = nc.dram_tensor("be", [128, 5], f32, kind="ExternalInput").ap()
    mb_d = nc.dram_tensor("mb", [128, 4], f32, kind="ExternalInput").ap()
    ob_d = nc.dram_tensor("ob", [128, 1], f32, kind="ExternalInput").ap()
    ones_d = nc.dram_tensor("ones16", [NG, C], f16, kind="ExternalInput").ap()
    out_d = nc.dram_tensor("out", [3, NPIX], f32, kind="ExternalOutput").ap()

    def mm2(psum_ap, lhsT_ap, rhs_ap):
        for h in range(C // MM):
            sl = slice(h * MM, (h + 1) * MM)
            nc.tensor.matmul(out=psum_ap[:, sl], lhsT=lhsT_ap,
                             rhs=rhs_ap[:, sl], start=True, stop=True)

    with tile.TileContext(nc) as tc:
        with (
            tc.tile_pool(name="consts", bufs=1) as cpool,
            tc.tile_pool(name="batch", bufs=1) as bpool,
            tc.tile_pool(name="ek", bufs=3) as ekpool,
            tc.tile_pool(name="sk", bufs=3) as skpool,
            tc.tile_pool(name="pq", bufs=2, space="PSUM") as pqpool,
            tc.tile_pool(name="psm", bufs=2, space="PSUM") as psmpool,
            tc.tile_pool(name="e", bufs=12) as epool,
            tc.tile_pool(name="sev", bufs=5) as sevpool,
            tc.tile_pool(name="g", bufs=22) as gpool,
            tc.tile_pool(name="o", bufs=6) as opool,
            tc.tile_pool(name="fin", bufs=3) as fpool,
        ):
            # xs first so the feature build starts ASAP
            BE = bpool.tile([NG, 5 * C], f32r)
            nc.sync.dma_start(
                out=BE[:, 0:C],
                in_=xs[0:1, :].rearrange("c (g p) -> (c g) p", p=C))
            nc.sync.dma_start(
                out=BE[:, C:2 * C],
                in_=xs[1:2, :].rearrange("c (g p) -> (c g) p", p=C))
            gabe = cpool.tile([10, 5 * 128], f32r)
            nc.sync.dma_start(out=gabe, in_=gabe_d)
            be = cpool.tile([128, 5], f32)
            nc.sync.dma_start(out=be, in_=be_d)
            sint = cpool.tile([2 * NM, 5 * 128], f16)
            nc.sync.dma_start(out=sint, in_=sint_d)
            mlpt = cpool.tile([128, 4 * 128], f16)
            nc.sync.dma_start(out=mlpt, in_=mlpt_d)
            fint = cpool.tile([128, 6], f16)
            nc.sync.dma_start(out=fint, in_=fint_d)
            mb = cpool.tile([128, 4], f32)
            nc.sync.dma_start(out=mb, in_=mb_d)
            ob = cpool.tile([128, 1], f32)
            nc.sync.dma_start(out=ob, in_=ob_d)

            # ---- one-time per-core feature build --------------------------
            # BE [64, 5*C] f32r: squares on ACT, cross term on DVE
            nc.scalar.activation(out=BE[:, 2 * C:3 * C], in_=BE[:, 0:C],
                                 func=AF.Square, bias=0.0)
            nc.vector.tensor_mul(out=BE[:, 3 * C:4 * C], in0=BE[:, 0:C],
                                 in1=BE[:, C:2 * C])
            nc.scalar.activation(out=BE[:, 4 * C:5 * C], in_=BE[:, C:2 * C],
                                 func=AF.Square, bias=0.0)
            BM = bpool.tile([NG, NM * C], f16)
            nc.sync.dma_start(out=BM[:, 0:C], in_=ones_d)
            # fp16 base: copy x1,x2; squares/cross recomputed in fp16 (2x DVE)
            nc.vector.tensor_copy(out=BM[:, C:3 * C], in_=BE[:, 0:2 * C])

            def build_sq_base():
                nc.vector.tensor_mul(out=BM[:, 3 * C:4 * C],
                                     in0=BM[:, C:2 * C], in1=BM[:, C:2 * C])
                nc.vector.tensor_mul(out=BM[:, 4 * C:5 * C],
                                     in0=BM[:, C:2 * C],
                                     in1=BM[:, 2 * C:3 * C])
                nc.vector.tensor_mul(out=BM[:, 5 * C:6 * C],
                                     in0=BM[:, 2 * C:3 * C],
                                     in1=BM[:, 2 * C:3 * C])

            def build_monos():
                sq_map = {(2, 2): (1, 1), (4, 0): (2, 0), (0, 4): (0, 2)}
                nmono = 0
                for d in range(3, DEG + 1):
                    for i in range(d, -1, -1):
                        j = d - i
                        if (i, j) in sq_map:
                            src = MIDX[sq_map[(i, j)]]
                            dst = MIDX[(i, j)]
                            nc.scalar.activation(
                                out=BM[:, dst * C:(dst + 1) * C],
                                in_=BM[:, src * C:(src + 1) * C],
                                func=AF.Square, bias=0.0)
                            continue
                        dst = MIDX[(i, j)]
                        if i > 0:
                            src, mul = MIDX[(i - 1, j)], 1
                        else:
                            src, mul = MIDX[(i, j - 1)], 2
                        eng = nc.gpsimd if nmono % 5 == 4 else nc.vector
                        eng.tensor_mul(out=BM[:, dst * C:(dst + 1) * C],
                                       in0=BM[:, src * C:(src + 1) * C],
                                       in1=BM[:, mul * C:(mul + 1) * C])
                        nmono += 1

            # ---- main loop: modulo software pipeline ----------------------
            # iteration `it` emits, in this order:
            #   gathers(it+1) | q/exp(it) | s + sin-combine(it-1)
            #   | mlp stage l of tile it-1-l (l=1..4) | fin(it-6)
            # so every instruction's dependencies are >= 1 iteration old and
            # no engine sequencer head-of-line-blocks on a late dependency.
            eK = {}   # t -> expK tile
            sK = {}   # t -> sinK tile
            es = {}   # t -> [e_l]
            gs = {}   # t -> [g_l]
            outs = {} # t -> cur (latest out_l)

            def gatherE(t):
                expK = ekpool.tile([10, C], f32r, tag="ek")
                nc.sync.dma_start(
                    out=expK,
                    in_=BE[2 * t:2 * t + 2].rearrange("p (f c) -> p f c", c=C))
                eK[t] = expK

            def gatherS(t):
                sinK = skpool.tile([2 * NM, C], f16, tag="sk")
                nc.sync.dma_start(
                    out=sinK,
                    in_=BM[2 * t:2 * t + 2].rearrange("p (m c) -> p m c", c=C))
                sK[t] = sinK

            def gather(t):
                gatherE(t)
                gatherS(t)

            psQs = {}

            def qmm(t):
                psQs[t] = []
                for l in range(5):
                    psQ = pqpool.tile([128, C], f32, tag="q")
                    mm2(psQ, gabe[:, l * 128:(l + 1) * 128], eK[t])
                    psQs[t].append(psQ)
                del eK[t]

            def qexp(t):
                es[t] = []
                for l in range(5):
                    e = epool.tile([128, C], f16, tag="e")
                    nc.scalar.activation(out=e, in_=psQs[t][l], func=AF.Exp,
                                         bias=be[:, l:l + 1])
                    es[t].append(e)
                del psQs[t]

            def qpart(t):
                qmm(t)
                qexp(t)

            def spart(t):
                gs[t] = [None] * 5
                for l in (2, 0, 3, 1, 4):
                    route = sin_route(t, l)
                    pool = pqpool if route != 'dve' else psmpool
                    psS = pool.tile([128, C], f32,
                                    tag="q" if route != 'dve' else "sm")
                    mm2(psS, sint[:, l * 128:(l + 1) * 128], sK[t])
                    e = es[t][l]
                    g = gpool.tile([128, C], f16, tag="g")
                    if route == 'dve':
                        nc.vector.tensor_mul(out=g, in0=psS, in1=e)
                    else:
                        ssb = sevpool.tile([128, C], f16, tag="sev")
                        nc.scalar.activation(out=ssb, in_=psS,
                                             func=AF.Identity, bias=0.0)
                        nc.gpsimd.tensor_mul(out=g, in0=ssb, in1=e)
                    gs[t][l] = g
                del es[t], sK[t]
                outs[t] = gs[t][0]

            def mpart(t, l):
                psM = psmpool.tile([128, C], f32, tag="sm")
                mm2(psM, mlpt[:, (l - 1) * 128:l * 128], outs[t])
                nxt = opool.tile([128, C], f16, tag="o")
                mroute = mlp_route(t, l)
                if mroute == 'dve':
                    nc.vector.scalar_tensor_tensor(
                        out=nxt, in0=psM, scalar=mb[:, l - 1:l], in1=gs[t][l],
                        op0=ALU.add, op1=ALU.mult)
                else:
                    msb = sevpool.tile([128, C], f16, tag="sev")
                    nc.scalar.activation(out=msb, in_=psM, func=AF.Identity,
                                         bias=mb[:, l - 1:l])
                    if mroute == 'act_pool':
                        nc.gpsimd.tensor_mul(out=nxt, in0=msb, in1=gs[t][l])
                    else:
                        nc.vector.tensor_mul(out=nxt, in0=msb, in1=gs[t][l])
                outs[t] = nxt

            fsbs = {}

            def finpart(t):
                psF = psmpool.tile([128, C], f32, tag="sm")
                mm2(psF[0:6], fint, outs[t])
                fsb = fpool.tile([6, C], f32, tag="f")
                if fin_route(t) == 'act':
                    nc.scalar.activation(out=fsb, in_=psF[0:6],
                                         func=AF.Identity, bias=ob[0:6])
                else:
                    nc.vector.tensor_scalar(out=fsb, in0=psF[0:6],
                                            scalar1=ob[0:6], scalar2=None,
                                            op0=ALU.add)
                fsbs[t] = fsb
                del gs[t], outs[t]

            def outdma(t):
                fsb = fsbs.pop(t)
                nc.sync.dma_start(out=out_d[:, 2 * t * C:(2 * t + 1) * C],
                                  in_=fsb[0:3])
                nc.sync.dma_start(out=out_d[:, (2 * t + 1) * C:(2 * t + 2) * C],
                                  in_=fsb[3:6])

            # prologue: tile 0 q-part overlaps the monomial build
            # schedule tables: normal tiles spread mparts over 4 iterations;
            # the last TAILC tiles compress to 2/iteration to shorten drain
            msch = {}   # it -> [(t, l), ...]
            fsch = {}   # it -> [t, ...]
            dsch = {}   # it -> [t, ...]
            for t in range(NT):
                if t < NT - TAILC:
                    for l in range(1, 5):
                        msch.setdefault(t + 1 + l, []).append((t, l))
                    fsch.setdefault(t + 6, []).append(t)
                    dsch.setdefault(t + 7, []).append(t)
                else:
                    msch.setdefault(t + 2, []).extend([(t, 1), (t, 2)])
                    msch.setdefault(t + 3, []).extend([(t, 3), (t, 4)])
                    fsch.setdefault(t + 4, []).append(t)
                    dsch.setdefault(t + 5, []).append(t)
            last_it = max(max(msch), max(fsch), max(dsch))

            build_sq_base()
            gatherE(0)
            qpart(0)
            build_monos()
            gatherS(0)
            gatherE(1)
            gatherS(1)
            qmm(1)
            for it in range(1, last_it + 1):
                for t in dsch.get(it, ()):
                    outdma(t)
                if it + 1 < NT:
                    gatherE(it + 1)
                    gatherS(it + 1)
                if it < NT:
                    qexp(it)
                if 0 <= it - 1 < NT:
                    spart(it - 1)
                for t, l in msch.get(it, ()):
                    mpart(t, l)
                for t in fsch.get(it, ()):
                    finpart(t)
                if it + 1 < NT:
                    qmm(it + 1)
    nc.compile()
    return nc


def _get_nc():
    if "nc" not in _CACHE:
        _CACHE["nc"] = _build_nc()
    return _CACHE["nc"]


def _in_maps(x, consts):
    maps = []
    rows = H // (NCORES // B)  # 128 rows per core
    for k in range(NCORES):
        b, r = k // (NCORES // B), (k % (NCORES // B)) * rows
        m = {"xs": np.ascontiguousarray(
            x[b, :, r:r + rows, :].reshape(2, NPIX), np.float32)}
        m.update(consts)
        maps.append(m)
    return maps


def _assemble(results):
    rows = H // (NCORES // B)
    out = np.empty((B, OUT, H, W), np.float32)
    for k in range(NCORES):
        b, r = k // (NCORES // B), (k % (NCORES // B)) * rows
        out[b, :, r:r + rows, :] = results[k]["out"].reshape(OUT, rows, W)
    return out


def run(x, filt_w, filt_b, mu, gamma, theta, lin_w, lin_b, out_w, out_b,
        trace=False):
    from concourse.bass_utils import run_bass_kernel_spmd
    nc = _get_nc()
    consts = _build_consts(np.asarray(filt_w), np.asarray(filt_b),
                           np.asarray(mu), np.asarray(gamma),
                           np.asarray(theta), np.asarray(lin_w),
                           np.asarray(lin_b), np.asarray(out_w),
                           np.asarray(out_b))
    maps = _in_maps(np.asarray(x), consts)
    res = run_bass_kernel_spmd(nc, maps, core_ids=list(range(NCORES)),
                               trace=trace)
    return _assemble(res.results), res


def kernel(**inputs):
    out, _ = run(**inputs)
    return out



# revision 6
# speedup vs baseline: 1.0502x; 1.0025x over previous
"""GaborNet Trainium2 kernel, v2.

Math per pixel p=(x1,x2), layer l, channel c:
  q_lc(p) = -0.5*||diag(gamma) R (p-mu)||^2   (quadratic in x1,x2)
  s_lc(p) = sin(filt_w . p + filt_b)
  out_0 = exp(q_0)*s_0;  out_l = exp(q_l)*s_l*(W_{l-1} out_{l-1} + b_{l-1})
  final = out_w @ out_4 (+ out_b, applied host-side)

Key structure vs v1:
  * sin is a PE matmul: sin(w.p+b) expanded in a Taylor series around b
    (|w.p| <~ 0.8) into monomials x1^i x2^j of degree <= D=6.  The constant
    term rides in a ones-feature row, so psum_s holds sin() directly and the
    scalar engine never runs Sin -> only Exp+Identity -> ONE act table, zero
    table switches.
  * per-pixel features (exp quadratic feats fp32, sin monomials fp16) are
    built once per core in batch layout [64 groups x slot*C], then gathered
    per tile into matmul K-blocks with one DMA each.
  * fp16 for all SBUF value tensors (DVE 2x mode); fp32r only on the exp-arg
    matmul path for precision.
  * the 9 combine units per tile are routed across DVE (psum TT / stt),
    ACT (identity evac + DVE/Pool mul) and GPSIMD to balance all engines.

Layout: channels on partitions, pixels on free dim; tiles of T=2048 pixels
as 2 groups of C=1024 stacked on partitions (64ch x 2 groups), block-diag
lhsT packing.  Sharding: 8 cores x 65536 consecutive pixels.
"""

import math

import numpy as np

B, DIM, H, W = 2, 2, 512, 512
HID, OUT, NL = 64, 3, 4
NCORES = 8
NPIX = B * H * W // NCORES  # 65536 pixels per core
C = 1024                    # columns per group
NG = NPIX // C              # 64 groups per core
NT = NG // 2                # 32 tiles (2 groups each)
MM = 512                    # psum-bank limit on matmul moving dim
DEG = 5                     # sin Taylor degree

# monomial slot order: (i, j) = x1^i * x2^j.
# slots 0..5 fixed: 1, x1, x2, x1^2, x1*x2, x2^2  (matches exp-feat order
# x1,x2,x1^2,x1x2,x2^2 at slots 1..5 for a contiguous BE->BM copy).
def _monos(deg):
    ms = [(0, 0), (1, 0), (0, 1), (2, 0), (1, 1), (0, 2)]
    for d in range(3, deg + 1):
        for i in range(d, -1, -1):
            ms.append((i, d - i))
    return ms

MONOS = _monos(DEG)
NM = len(MONOS)             # 28
MIDX = {m: k for k, m in enumerate(MONOS)}

# route config (tunable): per-layer sin route and mlp route
#   'dve'      sin: g = TT(psum_s, e)        mlp: out = stt(psum_m,+mb,*g)
#   'act_pool' ACT identity evac -> Pool mul
#   'act_dve'  ACT identity evac -> DVE mul
TAILC = 0                   # tiles with compressed mpart schedule at the end


def sin_route(t, l):
    """Ramp tiles keep ACT lean (exps only); steady evacuates l=2,3,4."""
    if l in (0, 1):
        return 'dve'
    if t < 3:
        return 'dve'
    if t == 3:
        return 'act_pool' if l == 4 else 'dve'
    if t == 4:
        return 'act_pool' if l in (3, 4) else 'dve'
    return 'act_pool'


def mlp_route(t, l):
    return 'dve'


def fin_route(t):
    return 'act' if t >= NT - 9 else 'dve'

_CACHE = {}


def _gabor_coeffs(filt_w, filt_b, mu, gamma, theta):
    """Exp-arg quadratic coeffs on feats [x1,x2,x1^2,x1x2,x2^2] + bias."""
    NL1 = theta.shape[0]
    Ge = np.zeros((NL1, 5, HID), np.float64)
    be = np.zeros((NL1, HID), np.float64)
    for l in range(NL1):
        ang = 2.0 * np.pi * theta[l].astype(np.float64)
        c, s = np.cos(ang), np.sin(ang)
        R = np.stack([np.stack([c, s], -1), np.stack([-s, c], -1)], -2)
        A = gamma[l].astype(np.float64)[:, :, None] * R
        Q = np.einsum('coi,coj->cij', A, A)
        Qmu = np.einsum('cij,cj->ci', Q, mu[l].astype(np.float64))
        Ge[l, 0] = Qmu[:, 0]
        Ge[l, 1] = Qmu[:, 1]
        Ge[l, 2] = -0.5 * Q[:, 0, 0]
        Ge[l, 3] = -Q[:, 0, 1]
        Ge[l, 4] = -0.5 * Q[:, 1, 1]
        be[l] = -0.5 * np.einsum('ci,ci->c', mu[l].astype(np.float64), Qmu)
    return Ge, be


def _sin_poly(filt_w, filt_b):
    """Per layer: [NM, HID] monomial coeffs of sin(w.p + b), Taylor deg DEG."""
    NL1 = filt_b.shape[0]
    P = np.zeros((NL1, NM, HID), np.float64)
    w = filt_w.astype(np.float64)
    b = filt_b.astype(np.float64)
    umax = np.max(np.abs(w[:, :, 0]) + np.abs(w[:, :, 1]))
    assert umax < 1.3, f"sin Taylor deg {DEG} insufficient for |u|max={umax}"
    for l in range(NL1):
        for k in range(DEG + 1):
            dk = np.sin(b[l] + k * np.pi / 2.0) / math.factorial(k)  # [HID]
            for j in range(k + 1):
                m = MIDX[(j, k - j)]
                P[l, m] += dk * math.comb(k, j) * w[l, :, 0] ** j * w[l, :, 1] ** (k - j)
    return P


def _build_consts(filt_w, filt_b, mu, gamma, theta, lin_w, lin_b, out_w, out_b):
    Ge, be = _gabor_coeffs(filt_w, filt_b, mu, gamma, theta)
    P = _sin_poly(filt_w, filt_b)
    NL1 = NL + 1
    # exp lhsT blocks [10, 5*128] f32: rows 0-4 grpA feats, 5-9 grpB
    gabe = np.zeros((10, NL1 * 128), np.float32)
    for l in range(NL1):
        gabe[0:5, l * 128:l * 128 + 64] = Ge[l]
        gabe[5:10, l * 128 + 64:l * 128 + 128] = Ge[l]
    # sin lhsT blocks [2*NM, 5*128] f16
    sint = np.zeros((2 * NM, NL1 * 128), np.float16)
    for l in range(NL1):
        sint[0:NM, l * 128:l * 128 + 64] = P[l]
        sint[NM:2 * NM, l * 128 + 64:l * 128 + 128] = P[l]
    # mlp lhsT [128, 4*128] f16 block-diag W^T
    mlpt = np.zeros((128, NL * 128), np.float16)
    for l in range(NL):
        wT = lin_w[l].T
        mlpt[0:64, l * 128:l * 128 + 64] = wT
        mlpt[64:128, l * 128 + 64:l * 128 + 128] = wT
    # fin lhsT [128, 6] f16
    fint = np.zeros((128, 6), np.float16)
    fint[0:64, 0:3] = out_w.T
    fint[64:128, 3:6] = out_w.T
    beB = np.concatenate([be, be], axis=1).T.astype(np.float32)      # [128,5]
    mbB = np.concatenate([lin_b, lin_b], axis=1).T.astype(np.float32)  # [128,4]
    ones16 = np.ones((NG, C), np.float16)
    obB = np.zeros((128, 1), np.float32)
    obB[0:3, 0] = out_b
    obB[3:6, 0] = out_b
    return dict(gabe=gabe, sint=sint, mlpt=mlpt, fint=fint, be=beB, mb=mbB,
                ones16=ones16, ob=obB)


def _build_nc():
    import concourse.mybir as mybir
    import concourse.tile as tile
    from concourse import bacc

    f32 = mybir.dt.float32
    f32r = mybir.dt.float32r
    f16 = mybir.dt.float16
    AF = mybir.ActivationFunctionType
    ALU = mybir.AluOpType

    nc = bacc.Bacc("TRN2", target_bir_lowering=False, debug=False,
                   enable_asserts=False, num_devices=NCORES)

    xs = nc.dram_tensor("xs", [2, NPIX], f32r, kind="ExternalInput").ap()
    gabe_d = nc.dram_tensor("gabe", [10, 5 * 128], f32r, kind="ExternalInput").ap()
    sint_d = nc.dram_tensor("sint", [2 * NM, 5 * 128], f16, kind="ExternalInput").ap()
    mlpt_d = nc.dram_tensor("mlpt", [128, 4 * 128], f16, kind="ExternalInput").ap()
    fint_d = nc.dram_tensor("fint", [128, 6], f16, kind="ExternalInput").ap()
    be_d = nc.dram_tensor("be", [128, 5], f32, kind="ExternalInput").ap()
    mb_d = nc.dram_tensor("mb", [128, 4], f32, kind="ExternalInput").ap()
    ob_d = nc.dram_tensor("ob", [128, 1], f32, kind="ExternalInput").ap()
    ones_d = nc.dram_tensor("ones16", [NG, C], f16, kind="ExternalInput").ap()
    out_d = nc.dram_tensor("out", [3, NPIX], f32, kind="ExternalOutput").ap()

    def mm2(psum_ap, lhsT_ap, rhs_ap):
        for h in range(C // MM):
            sl = slice(h * MM, (h + 1) * MM)
            nc.tensor.matmul(out=psum_ap[:, sl], lhsT=lhsT_ap,
                             rhs=rhs_ap[:, sl], start=True, stop=True)

    with tile.TileContext(nc) as tc:
        with (
            tc.tile_pool(name="consts", bufs=1) as cpool,
            tc.tile_pool(name="batch", bufs=1) as bpool,
            tc.tile_pool(name="ek", bufs=3) as ekpool,
            tc.tile_pool(name="sk", bufs=3) as skpool,
            tc.tile_pool(name="pq", bufs=2, space="PSUM") as pqpool,
            tc.tile_pool(name="psm", bufs=2, space="PSUM") as psmpool,
            tc.tile_pool(name="e", bufs=12) as epool,
            tc.tile_pool(name="sev", bufs=5) as sevpool,
            tc.tile_pool(name="g", bufs=22) as gpool,
            tc.tile_pool(name="o", bufs=6) as opool,
            tc.tile_pool(name="fin", bufs=3) as fpool,
        ):
            # xs first so the feature build starts ASAP
            BE = bpool.tile([NG, 5 * C], f32r)
            nc.sync.dma_start(
                out=BE[:, 0:C],
                in_=xs[0:1, :].rearrange("c (g p) -> (c g) p", p=C))
            nc.sync.dma_start(
                out=BE[:, C:2 * C],
                in_=xs[1:2, :].rearrange("c (g p) -> (c g) p", p=C))
            gabe = cpool.tile([10, 5 * 128], f32r)
            nc.sync.dma_start(out=gabe, in_=gabe_d)
            be = cpool.tile([128, 5], f32)
            nc.sync.dma_start(out=be, in_=be_d)
            sint = cpool.tile([2 * NM, 5 * 128], f16)
            nc.sync.dma_start(out=sint, in_=sint_d)
            mlpt = cpool.tile([128, 4 * 128], f16)
            nc.sync.dma_start(out=mlpt, in_=mlpt_d)
            fint = cpool.tile([128, 6], f16)
            nc.sync.dma_start(out=fint, in_=fint_d)
            mb = cpool.tile([128, 4], f32)
            nc.sync.dma_start(out=mb, in_=mb_d)
            ob = cpool.tile([128, 1], f32)
            nc.sync.dma_start(out=ob, in_=ob_d)

            # ---- one-time per-core feature build --------------------------
            # BE [64, 5*C] f32r: squares on ACT, cross term on DVE
            nc.scalar.activation(out=BE[:, 2 * C:3 * C], in_=BE[:, 0:C],
                                 func=AF.Square, bias=0.0)
            nc.vector.tensor_mul(out=BE[:, 3 * C:4 * C], in0=BE[:, 0:C],
                                 in1=BE[:, C:2 * C])
            nc.scalar.activation(out=BE[:, 4 * C:5 * C], in_=BE[:, C:2 * C],
                                 func=AF.Square, bias=0.0)
            BM = bpool.tile([NG, NM * C], f16)
            nc.sync.dma_start(out=BM[:, 0:C], in_=ones_d)
            # fp16 base: copy x1,x2; squares/cross recomputed in fp16 (2x DVE)
            nc.vector.tensor_copy(out=BM[:, C:2 * C], in_=BE[:, 0:C])
            nc.scalar.activation(out=BM[:, 2 * C:3 * C], in_=BE[:, C:2 * C],
                                 func=AF.Identity, bias=0.0)

            def build_sq_base():
                nc.vector.tensor_mul(out=BM[:, 3 * C:4 * C],
                                     in0=BM[:, C:2 * C], in1=BM[:, C:2 * C])
                nc.vector.tensor_mul(out=BM[:, 4 * C:5 * C],
                                     in0=BM[:, C:2 * C],
                                     in1=BM[:, 2 * C:3 * C])
                nc.vector.tensor_mul(out=BM[:, 5 * C:6 * C],
                                     in0=BM[:, 2 * C:3 * C],
                                     in1=BM[:, 2 * C:3 * C])

            def build_monos():
                sq_map = {(2, 2): (1, 1), (4, 0): (2, 0), (0, 4): (0, 2)}
                nmono = 0
                for d in range(3, DEG + 1):
                    for i in range(d, -1, -1):
                        j = d - i
                        if (i, j) in sq_map:
                            src = MIDX[sq_map[(i, j)]]
                            dst = MIDX[(i, j)]
                            nc.scalar.activation(
                                out=BM[:, dst * C:(dst + 1) * C],
                                in_=BM[:, src * C:(src + 1) * C],
                                func=AF.Square, bias=0.0)
                            continue
                        dst = MIDX[(i, j)]
                        if i > 0:
                            src, mul = MIDX[(i - 1, j)], 1
                        else:
                            src, mul = MIDX[(i, j - 1)], 2
                        eng = nc.gpsimd if nmono % 5 == 4 else nc.vector
                        eng.tensor_mul(out=BM[:, dst * C:(dst + 1) * C],
                                       in0=BM[:, src * C:(src + 1) * C],
                                       in1=BM[:, mul * C:(mul + 1) * C])
                        nmono += 1

            # ---- main loop: modulo software pipeline ----------------------
            # iteration `it` emits, in this order:
            #   gathers(it+1) | q/exp(it) | s + sin-combine(it-1)
            #   | mlp stage l of tile it-1-l (l=1..4) | fin(it-6)
            # so every instruction's dependencies are >= 1 iteration old and
            # no engine sequencer head-of-line-blocks on a late dependency.
            eK = {}   # t -> expK tile
            sK = {}   # t -> sinK tile
            es = {}   # t -> [e_l]
            gs = {}   # t -> [g_l]
            outs = {} # t -> cur (latest out_l)

            def gatherE(t):
                expK = ekpool.tile([10, C], f32r, tag="ek")
                nc.sync.dma_start(
                    out=expK,
                    in_=BE[2 * t:2 * t + 2].rearrange("p (f c) -> p f c", c=C))
                eK[t] = expK

            def gatherS(t):
                sinK = skpool.tile([2 * NM, C], f16, tag="sk")
                nc.sync.dma_start(
                    out=sinK,
                    in_=BM[2 * t:2 * t + 2].rearrange("p (m c) -> p m c", c=C))
                sK[t] = sinK

            def gather(t):
                gatherE(t)
                gatherS(t)

            psQs = {}

            def qmm(t):
                psQs[t] = []
                for l in range(5):
                    psQ = pqpool.tile([128, C], f32, tag="q")
                    mm2(psQ, gabe[:, l * 128:(l + 1) * 128], eK[t])
                    psQs[t].append(psQ)
                del eK[t]

            def qexp(t):
                es[t] = []
                for l in range(5):
                    e = epool.tile([128, C], f16, tag="e")
                    nc.scalar.activation(out=e, in_=psQs[t][l], func=AF.Exp,
                                         bias=be[:, l:l + 1])
                    es[t].append(e)
                del psQs[t]

            def qpart(t):
                qmm(t)
                qexp(t)

            def spart(t):
                gs[t] = [None] * 5
                for l in (2, 0, 3, 1, 4):
                    route = sin_route(t, l)
                    pool = pqpool if route != 'dve' else psmpool
                    psS = pool.tile([128, C], f32,
                                    tag="q" if route != 'dve' else "sm")
                    mm2(psS, sint[:, l * 128:(l + 1) * 128], sK[t])
                    e = es[t][l]
                    g = gpool.tile([128, C], f16, tag="g")
                    if route == 'dve':
                        nc.vector.tensor_mul(out=g, in0=psS, in1=e)
                    else:
                        ssb = sevpool.tile([128, C], f16, tag="sev")
                        nc.scalar.activation(out=ssb, in_=psS,
                                             func=AF.Identity, bias=0.0)
                        nc.gpsimd.tensor_mul(out=g, in0=ssb, in1=e)
                    gs[t][l] = g
                del es[t], sK[t]
                outs[t] = gs[t][0]

            def mpart(t, l):
                psM = psmpool.tile([128, C], f32, tag="sm")
                mm2(psM, mlpt[:, (l - 1) * 128:l * 128], outs[t])
                nxt = opool.tile([128, C], f16, tag="o")
                mroute = mlp_route(t, l)
                if mroute == 'dve':
                    nc.vector.scalar_tensor_tensor(
                        out=nxt, in0=psM, scalar=mb[:, l - 1:l], in1=gs[t][l],
                        op0=ALU.add, op1=ALU.mult)
                else:
                    msb = sevpool.tile([128, C], f16, tag="sev")
                    nc.scalar.activation(out=msb, in_=psM, func=AF.Identity,
                                         bias=mb[:, l - 1:l])
                    if mroute == 'act_pool':
                        nc.gpsimd.tensor_mul(out=nxt, in0=msb, in1=gs[t][l])
                    else:
                        nc.vector.tensor_mul(out=nxt, in0=msb, in1=gs[t][l])
                outs[t] = nxt

            fsbs = {}

            def finpart(t):
                psF = psmpool.tile([128, C], f32, tag="sm")
                mm2(psF[0:6], fint, outs[t])
                fsb = fpool.tile([6, C], f32, tag="f")
                if fin_route(t) == 'act':
                    nc.scalar.activation(out=fsb, in_=psF[0:6],
                                         func=AF.Identity, bias=ob[0:6])
                else:
                    nc.vector.tensor_scalar(out=fsb, in0=psF[0:6],
                                            scalar1=ob[0:6], scalar2=None,
                                            op0=ALU.add)
                fsbs[t] = fsb
                del gs[t], outs[t]

            def outdma(t):
                fsb = fsbs.pop(t)
                nc.sync.dma_start(out=out_d[:, 2 * t * C:(2 * t + 1) * C],
                                  in_=fsb[0:3])
                nc.sync.dma_start(out=out_d[:, (2 * t + 1) * C:(2 * t + 2) * C],
                                  in_=fsb[3:6])

            # prologue: tile 0 q-part overlaps the monomial build
            # schedule tables: normal tiles spread mparts over 4 iterations;
            # the last TAILC tiles compress to 2/iteration to shorten drain
            msch = {}   # it -> [(t, l), ...]
            fsch = {}   # it -> [t, ...]
            dsch = {}   # it -> [t, ...]
            for t in range(NT):
                if t < NT - TAILC:
                    for l in range(1, 5):
                        msch.setdefault(t + 1 + l, []).append((t, l))
                    fsch.setdefault(t + 6, []).append(t)
                    dsch.setdefault(t + 7, []).append(t)
                else:
                    msch.setdefault(t + 2, []).extend([(t, 1), (t, 2)])
                    msch.setdefault(t + 3, []).extend([(t, 3), (t, 4)])
                    fsch.setdefault(t + 4, []).append(t)
                    dsch.setdefault(t + 5, []).append(t)
            last_it = max(max(msch), max(fsch), max(dsch))

            build_sq_base()
            gatherE(0)
            qpart(0)
            build_monos()
            gatherS(0)
            gatherE(1)
            gatherS(1)
            qmm(1)
            for it in range(1, last_it + 1):
                for t in dsch.get(it, ()):
                    outdma(t)
                if it + 1 < NT:
                    gatherE(it + 1)
                    gatherS(it + 1)
                if it < NT:
                    qexp(it)
                if 0 <= it - 1 < NT:
                    spart(it - 1)
                for t, l in msch.get(it, ()):
                    mpart(t, l)
                for t in fsch.get(it, ()):
                    finpart(t)
                if it + 1 < NT:
                    qmm(it + 1)
    nc.compile()
    return nc


def _get_nc():
    if "nc" not in _CACHE:
        _CACHE["nc"] = _build_nc()
    return _CACHE["nc"]


def _in_maps(x, consts):
    maps = []
    rows = H // (NCORES // B)  # 128 rows per core
    for k in range(NCORES):
        b, r = k // (NCORES // B), (k % (NCORES // B)) * rows
        m = {"xs": np.ascontiguousarray(
            x[b, :, r:r + rows, :].reshape(2, NPIX), np.float32)}
        m.update(consts)
        maps.append(m)
    return maps


def _assemble(results):
    rows = H // (NCORES // B)
    out = np.empty((B, OUT, H, W), np.float32)
    for k in range(NCORES):
        b, r = k // (NCORES // B), (k % (NCORES // B)) * rows
        out[b, :, r:r + rows, :] = results[k]["out"].reshape(OUT, rows, W)
    return out


def run(x, filt_w, filt_b, mu, gamma, theta, lin_w, lin_b, out_w, out_b,
        trace=False):
    from concourse.bass_utils import run_bass_kernel_spmd
    nc = _get_nc()
    consts = _build_consts(np.asarray(filt_w), np.asarray(filt_b),
                           np.asarray(mu), np.asarray(gamma),
                           np.asarray(theta), np.asarray(lin_w),
                           np.asarray(lin_b), np.asarray(out_w),
                           np.asarray(out_b))
    maps = _in_maps(np.asarray(x), consts)
    res = run_bass_kernel_spmd(nc, maps, core_ids=list(range(NCORES)),
                               trace=trace)
    return _assemble(res.results), res


def kernel(**inputs):
    out, _ = run(**inputs)
    return out



# revision 7
# speedup vs baseline: 1.0504x; 1.0002x over previous
"""GaborNet Trainium2 kernel, v2.

Math per pixel p=(x1,x2), layer l, channel c:
  q_lc(p) = -0.5*||diag(gamma) R (p-mu)||^2   (quadratic in x1,x2)
  s_lc(p) = sin(filt_w . p + filt_b)
  out_0 = exp(q_0)*s_0;  out_l = exp(q_l)*s_l*(W_{l-1} out_{l-1} + b_{l-1})
  final = out_w @ out_4 (+ out_b, applied host-side)

Key structure vs v1:
  * sin is a PE matmul: sin(w.p+b) expanded in a Taylor series around b
    (|w.p| <~ 0.8) into monomials x1^i x2^j of degree <= D=6.  The constant
    term rides in a ones-feature row, so psum_s holds sin() directly and the
    scalar engine never runs Sin -> only Exp+Identity -> ONE act table, zero
    table switches.
  * per-pixel features (exp quadratic feats fp32, sin monomials fp16) are
    built once per core in batch layout [64 groups x slot*C], then gathered
    per tile into matmul K-blocks with one DMA each.
  * fp16 for all SBUF value tensors (DVE 2x mode); fp32r only on the exp-arg
    matmul path for precision.
  * the 9 combine units per tile are routed across DVE (psum TT / stt),
    ACT (identity evac + DVE/Pool mul) and GPSIMD to balance all engines.

Layout: channels on partitions, pixels on free dim; tiles of T=2048 pixels
as 2 groups of C=1024 stacked on partitions (64ch x 2 groups), block-diag
lhsT packing.  Sharding: 8 cores x 65536 consecutive pixels.
"""

import math

import numpy as np

B, DIM, H, W = 2, 2, 512, 512
HID, OUT, NL = 64, 3, 4
NCORES = 8
NPIX = B * H * W // NCORES  # 65536 pixels per core
C = 1024                    # columns per group
NG = NPIX // C              # 64 groups per core
NT = NG // 2                # 32 tiles (2 groups each)
MM = 512                    # psum-bank limit on matmul moving dim
DEG = 5                     # sin Taylor degree

# monomial slot order: (i, j) = x1^i * x2^j.
# slots 0..5 fixed: 1, x1, x2, x1^2, x1*x2, x2^2  (matches exp-feat order
# x1,x2,x1^2,x1x2,x2^2 at slots 1..5 for a contiguous BE->BM copy).
def _monos(deg):
    ms = [(0, 0), (1, 0), (0, 1), (2, 0), (1, 1), (0, 2)]
    for d in range(3, deg + 1):
        for i in range(d, -1, -1):
            ms.append((i, d - i))
    return ms

MONOS = _monos(DEG)
NM = len(MONOS)             # 28
MIDX = {m: k for k, m in enumerate(MONOS)}

# route config (tunable): per-layer sin route and mlp route
#   'dve'      sin: g = TT(psum_s, e)        mlp: out = stt(psum_m,+mb,*g)
#   'act_pool' ACT identity evac -> Pool mul
#   'act_dve'  ACT identity evac -> DVE mul
TAILC = 0                   # tiles with compressed mpart schedule at the end


def sin_route(t, l):
    """Ramp tiles keep ACT lean (exps only); steady evacuates l=2,3,4."""
    if l in (0, 1):
        return 'dve'
    if t < 3:
        return 'dve'
    if t == 3:
        return 'act_pool' if l == 4 else 'dve'
    if t == 4:
        return 'act_pool' if l in (3, 4) else 'dve'
    return 'act_pool'


def mlp_route(t, l):
    return 'dve'


def fin_route(t):
    return 'act' if t >= NT - 9 else 'dve'

_CACHE = {}


def _gabor_coeffs(filt_w, filt_b, mu, gamma, theta):
    """Exp-arg quadratic coeffs on feats [x1,x2,x1^2,x1x2,x2^2] + bias."""
    NL1 = theta.shape[0]
    Ge = np.zeros((NL1, 5, HID), np.float64)
    be = np.zeros((NL1, HID), np.float64)
    for l in range(NL1):
        ang = 2.0 * np.pi * theta[l].astype(np.float64)
        c, s = np.cos(ang), np.sin(ang)
        R = np.stack([np.stack([c, s], -1), np.stack([-s, c], -1)], -2)
        A = gamma[l].astype(np.float64)[:, :, None] * R
        Q = np.einsum('coi,coj->cij', A, A)
        Qmu = np.einsum('cij,cj->ci', Q, mu[l].astype(np.float64))
        Ge[l, 0] = Qmu[:, 0]
        Ge[l, 1] = Qmu[:, 1]
        Ge[l, 2] = -0.5 * Q[:, 0, 0]
        Ge[l, 3] = -Q[:, 0, 1]
        Ge[l, 4] = -0.5 * Q[:, 1, 1]
        be[l] = -0.5 * np.einsum('ci,ci->c', mu[l].astype(np.float64), Qmu)
    return Ge, be


def _sin_poly(filt_w, filt_b):
    """Per layer: [NM, HID] monomial coeffs of sin(w.p + b), Taylor deg DEG."""
    NL1 = filt_b.shape[0]
    P = np.zeros((NL1, NM, HID), np.float64)
    w = filt_w.astype(np.float64)
    b = filt_b.astype(np.float64)
    umax = np.max(np.abs(w[:, :, 0]) + np.abs(w[:, :, 1]))
    assert umax < 1.3, f"sin Taylor deg {DEG} insufficient for |u|max={umax}"
    for l in range(NL1):
        for k in range(DEG + 1):
            dk = np.sin(b[l] + k * np.pi / 2.0) / math.factorial(k)  # [HID]
            for j in range(k + 1):
                m = MIDX[(j, k - j)]
                P[l, m] += dk * math.comb(k, j) * w[l, :, 0] ** j * w[l, :, 1] ** (k - j)
    return P


def _build_consts(filt_w, filt_b, mu, gamma, theta, lin_w, lin_b, out_w, out_b):
    Ge, be = _gabor_coeffs(filt_w, filt_b, mu, gamma, theta)
    P = _sin_poly(filt_w, filt_b)
    NL1 = NL + 1
    # exp lhsT blocks [10, 5*128] f32: rows 0-4 grpA feats, 5-9 grpB
    gabe = np.zeros((10, NL1 * 128), np.float32)
    for l in range(NL1):
        gabe[0:5, l * 128:l * 128 + 64] = Ge[l]
        gabe[5:10, l * 128 + 64:l * 128 + 128] = Ge[l]
    # sin lhsT blocks [2*NM, 5*128] f16
    sint = np.zeros((2 * NM, NL1 * 128), np.float16)
    for l in range(NL1):
        sint[0:NM, l * 128:l * 128 + 64] = P[l]
        sint[NM:2 * NM, l * 128 + 64:l * 128 + 128] = P[l]
    # mlp lhsT [128, 4*128] f16 block-diag W^T
    mlpt = np.zeros((128, NL * 128), np.float16)
    for l in range(NL):
        wT = lin_w[l].T
        mlpt[0:64, l * 128:l * 128 + 64] = wT
        mlpt[64:128, l * 128 + 64:l * 128 + 128] = wT
    # fin lhsT [128, 6] f16
    fint = np.zeros((128, 6), np.float16)
    fint[0:64, 0:3] = out_w.T
    fint[64:128, 3:6] = out_w.T
    beB = np.concatenate([be, be], axis=1).T.astype(np.float32)      # [128,5]
    mbB = np.concatenate([lin_b, lin_b], axis=1).T.astype(np.float32)  # [128,4]
    ones16 = np.ones((NG, C), np.float16)
    obB = np.zeros((128, 1), np.float32)
    obB[0:3, 0] = out_b
    obB[3:6, 0] = out_b
    return dict(gabe=gabe, sint=sint, mlpt=mlpt, fint=fint, be=beB, mb=mbB,
                ones16=ones16, ob=obB)


def _build_nc():
    import concourse.mybir as mybir
    import concourse.tile as tile
    from concourse import bacc

    f32 = mybir.dt.float32
    f32r = mybir.dt.float32r
    f16 = mybir.dt.float16
    AF = mybir.ActivationFunctionType
    ALU = mybir.AluOpType

    nc = bacc.Bacc("TRN2", target_bir_lowering=False, debug=False,
                   enable_asserts=False, num_devices=NCORES)

    xs = nc.dram_tensor("xs", [2, NPIX], f32r, kind="ExternalInput").ap()
    gabe_d = nc.dram_tensor("gabe", [10, 5 * 128], f32r, kind="ExternalInput").ap()
    sint_d = nc.dram_tensor("sint", [2 * NM, 5 * 128], f16, kind="ExternalInput").ap()
    mlpt_d = nc.dram_tensor("mlpt", [128, 4 * 128], f16, kind="ExternalInput").ap()
    fint_d = nc.dram_tensor("fint", [128, 6], f16, kind="ExternalInput").ap()
    be_d = nc.dram_tensor("be", [128, 5], f32, kind="ExternalInput").ap()
    mb_d = nc.dram_tensor("mb", [128, 4], f32, kind="ExternalInput").ap()
    ob_d = nc.dram_tensor("ob", [128, 1], f32, kind="ExternalInput").ap()
    ones_d = nc.dram_tensor("ones16", [NG, C], f16, kind="ExternalInput").ap()
    out_d = nc.dram_tensor("out", [3, NPIX], f32, kind="ExternalOutput").ap()

    def mm2(psum_ap, lhsT_ap, rhs_ap):
        for h in range(C // MM):
            sl = slice(h * MM, (h + 1) * MM)
            nc.tensor.matmul(out=psum_ap[:, sl], lhsT=lhsT_ap,
                             rhs=rhs_ap[:, sl], start=True, stop=True)

    with tile.TileContext(nc) as tc:
        with (
            tc.tile_pool(name="consts", bufs=1) as cpool,
            tc.tile_pool(name="batch", bufs=1) as bpool,
            tc.tile_pool(name="ek", bufs=5) as ekpool,
            tc.tile_pool(name="sk", bufs=5) as skpool,
            tc.tile_pool(name="pq", bufs=2, space="PSUM") as pqpool,
            tc.tile_pool(name="psm", bufs=2, space="PSUM") as psmpool,
            tc.tile_pool(name="e", bufs=12) as epool,
            tc.tile_pool(name="sev", bufs=5) as sevpool,
            tc.tile_pool(name="g", bufs=22) as gpool,
            tc.tile_pool(name="o", bufs=6) as opool,
            tc.tile_pool(name="fin", bufs=3) as fpool,
        ):
            # xs first so the feature build starts ASAP
            BE = bpool.tile([NG, 5 * C], f32r)
            nc.sync.dma_start(
                out=BE[:, 0:C],
                in_=xs[0:1, :].rearrange("c (g p) -> (c g) p", p=C))
            nc.sync.dma_start(
                out=BE[:, C:2 * C],
                in_=xs[1:2, :].rearrange("c (g p) -> (c g) p", p=C))
            gabe = cpool.tile([10, 5 * 128], f32r)
            nc.sync.dma_start(out=gabe, in_=gabe_d)
            be = cpool.tile([128, 5], f32)
            nc.sync.dma_start(out=be, in_=be_d)
            sint = cpool.tile([2 * NM, 5 * 128], f16)
            nc.sync.dma_start(out=sint, in_=sint_d)
            mlpt = cpool.tile([128, 4 * 128], f16)
            nc.sync.dma_start(out=mlpt, in_=mlpt_d)
            fint = cpool.tile([128, 6], f16)
            nc.sync.dma_start(out=fint, in_=fint_d)
            mb = cpool.tile([128, 4], f32)
            nc.sync.dma_start(out=mb, in_=mb_d)
            ob = cpool.tile([128, 1], f32)
            nc.sync.dma_start(out=ob, in_=ob_d)

            # ---- one-time per-core feature build --------------------------
            # BE [64, 5*C] f32r: squares on ACT, cross term on DVE
            nc.scalar.activation(out=BE[:, 2 * C:3 * C], in_=BE[:, 0:C],
                                 func=AF.Square, bias=0.0)
            nc.vector.tensor_mul(out=BE[:, 3 * C:4 * C], in0=BE[:, 0:C],
                                 in1=BE[:, C:2 * C])
            nc.scalar.activation(out=BE[:, 4 * C:5 * C], in_=BE[:, C:2 * C],
                                 func=AF.Square, bias=0.0)
            BM = bpool.tile([NG, NM * C], f16)
            nc.sync.dma_start(out=BM[:, 0:C], in_=ones_d)
            # fp16 base: copy x1,x2; squares/cross recomputed in fp16 (2x DVE)
            nc.vector.tensor_copy(out=BM[:, C:2 * C], in_=BE[:, 0:C])
            nc.scalar.activation(out=BM[:, 2 * C:3 * C], in_=BE[:, C:2 * C],
                                 func=AF.Identity, bias=0.0)

            def build_sq_base():
                nc.vector.tensor_mul(out=BM[:, 3 * C:4 * C],
                                     in0=BM[:, C:2 * C], in1=BM[:, C:2 * C])
                nc.vector.tensor_mul(out=BM[:, 4 * C:5 * C],
                                     in0=BM[:, C:2 * C],
                                     in1=BM[:, 2 * C:3 * C])
                nc.vector.tensor_mul(out=BM[:, 5 * C:6 * C],
                                     in0=BM[:, 2 * C:3 * C],
                                     in1=BM[:, 2 * C:3 * C])

            def build_monos():
                sq_map = {(2, 2): (1, 1), (4, 0): (2, 0), (0, 4): (0, 2)}
                nmono = 0
                for d in range(3, DEG + 1):
                    for i in range(d, -1, -1):
                        j = d - i
                        if (i, j) in sq_map:
                            src = MIDX[sq_map[(i, j)]]
                            dst = MIDX[(i, j)]
                            nc.scalar.activation(
                                out=BM[:, dst * C:(dst + 1) * C],
                                in_=BM[:, src * C:(src + 1) * C],
                                func=AF.Square, bias=0.0)
                            continue
                        dst = MIDX[(i, j)]
                        if i > 0:
                            src, mul = MIDX[(i - 1, j)], 1
                        else:
                            src, mul = MIDX[(i, j - 1)], 2
                        eng = nc.gpsimd if nmono % 5 == 4 else nc.vector
                        eng.tensor_mul(out=BM[:, dst * C:(dst + 1) * C],
                                       in0=BM[:, src * C:(src + 1) * C],
                                       in1=BM[:, mul * C:(mul + 1) * C])
                        nmono += 1

            # ---- main loop: modulo software pipeline ----------------------
            # iteration `it` emits, in this order:
            #   gathers(it+1) | q/exp(it) | s + sin-combine(it-1)
            #   | mlp stage l of tile it-1-l (l=1..4) | fin(it-6)
            # so every instruction's dependencies are >= 1 iteration old and
            # no engine sequencer head-of-line-blocks on a late dependency.
            eK = {}   # t -> expK tile
            sK = {}   # t -> sinK tile
            es = {}   # t -> [e_l]
            gs = {}   # t -> [g_l]
            outs = {} # t -> cur (latest out_l)

            def gatherE(t):
                expK = ekpool.tile([10, C], f32r, tag="ek")
                nc.sync.dma_start(
                    out=expK,
                    in_=BE[2 * t:2 * t + 2].rearrange("p (f c) -> p f c", c=C))
                eK[t] = expK

            def gatherS(t):
                sinK = skpool.tile([2 * NM, C], f16, tag="sk")
                nc.sync.dma_start(
                    out=sinK,
                    in_=BM[2 * t:2 * t + 2].rearrange("p (m c) -> p m c", c=C))
                sK[t] = sinK

            def gather(t):
                gatherE(t)
                gatherS(t)

            psQs = {}

            def qmm(t):
                psQs[t] = []
                for l in range(5):
                    psQ = pqpool.tile([128, C], f32, tag="q")
                    mm2(psQ, gabe[:, l * 128:(l + 1) * 128], eK[t])
                    psQs[t].append(psQ)
                del eK[t]

            def qexp(t):
                es[t] = []
                for l in range(5):
                    e = epool.tile([128, C], f16, tag="e")
                    nc.scalar.activation(out=e, in_=psQs[t][l], func=AF.Exp,
                                         bias=be[:, l:l + 1])
                    es[t].append(e)
                del psQs[t]

            def qpart(t):
                qmm(t)
                qexp(t)

            def spart(t):
                gs[t] = [None] * 5
                for l in (2, 0, 3, 1, 4):
                    route = sin_route(t, l)
                    pool = pqpool if route != 'dve' else psmpool
                    psS = pool.tile([128, C], f32,
                                    tag="q" if route != 'dve' else "sm")
                    mm2(psS, sint[:, l * 128:(l + 1) * 128], sK[t])
                    e = es[t][l]
                    g = gpool.tile([128, C], f16, tag="g")
                    if route == 'dve':
                        nc.vector.tensor_mul(out=g, in0=psS, in1=e)
                    else:
                        ssb = sevpool.tile([128, C], f16, tag="sev")
                        nc.scalar.activation(out=ssb, in_=psS,
                                             func=AF.Identity, bias=0.0)
                        nc.gpsimd.tensor_mul(out=g, in0=ssb, in1=e)
                    gs[t][l] = g
                del es[t], sK[t]
                outs[t] = gs[t][0]

            def mpart(t, l):
                psM = psmpool.tile([128, C], f32, tag="sm")
                mm2(psM, mlpt[:, (l - 1) * 128:l * 128], outs[t])
                nxt = opool.tile([128, C], f16, tag="o")
                mroute = mlp_route(t, l)
                if mroute == 'dve':
                    nc.vector.scalar_tensor_tensor(
                        out=nxt, in0=psM, scalar=mb[:, l - 1:l], in1=gs[t][l],
                        op0=ALU.add, op1=ALU.mult)
                else:
                    msb = sevpool.tile([128, C], f16, tag="sev")
                    nc.scalar.activation(out=msb, in_=psM, func=AF.Identity,
                                         bias=mb[:, l - 1:l])
                    if mroute == 'act_pool':
                        nc.gpsimd.tensor_mul(out=nxt, in0=msb, in1=gs[t][l])
                    else:
                        nc.vector.tensor_mul(out=nxt, in0=msb, in1=gs[t][l])
                outs[t] = nxt

            fsbs = {}

            def finpart(t):
                psF = psmpool.tile([128, C], f32, tag="sm")
                mm2(psF[0:6], fint, outs[t])
                fsb = fpool.tile([6, C], f32, tag="f")
                if fin_route(t) == 'act':
                    nc.scalar.activation(out=fsb, in_=psF[0:6],
                                         func=AF.Identity, bias=ob[0:6])
                else:
                    nc.vector.tensor_scalar(out=fsb, in0=psF[0:6],
                                            scalar1=ob[0:6], scalar2=None,
                                            op0=ALU.add)
                fsbs[t] = fsb
                del gs[t], outs[t]

            def outdma(t):
                fsb = fsbs.pop(t)
                nc.sync.dma_start(out=out_d[:, 2 * t * C:(2 * t + 1) * C],
                                  in_=fsb[0:3])
                nc.sync.dma_start(out=out_d[:, (2 * t + 1) * C:(2 * t + 2) * C],
                                  in_=fsb[3:6])

            # prologue: tile 0 q-part overlaps the monomial build
            # schedule tables: normal tiles spread mparts over 4 iterations;
            # the last TAILC tiles compress to 2/iteration to shorten drain
            msch = {}   # it -> [(t, l), ...]
            fsch = {}   # it -> [t, ...]
            dsch = {}   # it -> [t, ...]
            for t in range(NT):
                if t < NT - TAILC:
                    for l in range(1, 5):
                        msch.setdefault(t + 1 + l, []).append((t, l))
                    fsch.setdefault(t + 6, []).append(t)
                    dsch.setdefault(t + 7, []).append(t)
                else:
                    msch.setdefault(t + 2, []).extend([(t, 1), (t, 2)])
                    msch.setdefault(t + 3, []).extend([(t, 3), (t, 4)])
                    fsch.setdefault(t + 4, []).append(t)
                    dsch.setdefault(t + 5, []).append(t)
            last_it = max(max(msch), max(fsch), max(dsch))

            build_sq_base()
            gatherE(0)
            qpart(0)
            build_monos()
            gatherS(0)
            gatherE(1)
            gatherS(1)
            qmm(1)
            for it in range(1, last_it + 1):
                for t in dsch.get(it, ()):
                    outdma(t)
                if it + 1 < NT:
                    gatherE(it + 1)
                    gatherS(it + 1)
                if it < NT:
                    qexp(it)
                if 0 <= it - 1 < NT:
                    spart(it - 1)
                for t, l in msch.get(it, ()):
                    mpart(t, l)
                for t in fsch.get(it, ()):
                    finpart(t)
                if it + 1 < NT:
                    qmm(it + 1)
    nc.compile()
    return nc


def _get_nc():
    if "nc" not in _CACHE:
        _CACHE["nc"] = _build_nc()
    return _CACHE["nc"]


def _in_maps(x, consts):
    maps = []
    rows = H // (NCORES // B)  # 128 rows per core
    for k in range(NCORES):
        b, r = k // (NCORES // B), (k % (NCORES // B)) * rows
        m = {"xs": np.ascontiguousarray(
            x[b, :, r:r + rows, :].reshape(2, NPIX), np.float32)}
        m.update(consts)
        maps.append(m)
    return maps


def _assemble(results):
    rows = H // (NCORES // B)
    out = np.empty((B, OUT, H, W), np.float32)
    for k in range(NCORES):
        b, r = k // (NCORES // B), (k % (NCORES // B)) * rows
        out[b, :, r:r + rows, :] = results[k]["out"].reshape(OUT, rows, W)
    return out


def run(x, filt_w, filt_b, mu, gamma, theta, lin_w, lin_b, out_w, out_b,
        trace=False):
    from concourse.bass_utils import run_bass_kernel_spmd
    nc = _get_nc()
    consts = _build_consts(np.asarray(filt_w), np.asarray(filt_b),
                           np.asarray(mu), np.asarray(gamma),
                           np.asarray(theta), np.asarray(lin_w),
                           np.asarray(lin_b), np.asarray(out_w),
                           np.asarray(out_b))
    maps = _in_maps(np.asarray(x), consts)
    res = run_bass_kernel_spmd(nc, maps, core_ids=list(range(NCORES)),
                               trace=trace)
    return _assemble(res.results), res


def kernel(**inputs):
    out, _ = run(**inputs)
    return out

